# revision 50
# baseline (speedup 1.0000x reference)
"""Trainium2 Bass kernel for nn_EnhancedEncoder (gnn_message_passing).

Data-parallel over the 1024 flattened groups: 128 groups per core on 8 cores.
All intermediates stay in SBUF. The KNN gather is reformulated with counts:
U[g] = sum_m (cnt_m - K) feat_m needs only per-position selection counts, so
the H loop has no neighbor-sum matmul on its critical chain; the neighbor sum
hps is kept only for the global T2 moment. fg2 comes from per-tile PE
transposes of position-major feat (conv4 is computed once). Weight/layout
transposes are done host-side in make_in_maps (pure relayout, no arithmetic).
Cross-core reductions: warm-up AllReduce + BN3 stats + one merged final round
(global std of dx + fusion BN stats); the KNN selection phase is declared
between the BN3 collective launch and its consumers so the in-order engine
queues execute it inside the collective's latency window.

Position indexing per core: pos = t*128 + gp*32 + i  (t in [0,32), gp in
[0,4), i in [0,32)); group id g = 4*t + gp.  Channel-major tensors are
[ch_tile(128), pos(4096)]; feat tiles are [4*32 points, 384]; per-group
vectors are [*, g] with g = 4*t + gp.
"""
import sys
from contextlib import ExitStack

for _p in ("/opt/trn_rl_repo",):
    if _p not in sys.path:
        sys.path.insert(0, _p)

import numpy as np

NCORES = 8
G = 128            # groups per core
NPTS = 32          # points per group
NPOS = G * NPTS    # 4096 positions per core
C = 384            # encoder channels
K = 8              # knn group size
EPS = 1e-5
NTOT_POS = 1024 * NPTS          # global positions (BN1/BN3 denominator)
NTOT_DX = 1024 * NPTS * K * C   # global dx element count (std denominator)
NB = 1024                       # global batch of groups (BNf denominator)
BIG_NEG = -1e30
DEBUG = False

_BUILT = None


def _build(debug=False):
    import concourse.bacc as bacc
    import concourse.tile as tile
    from concourse import mybir

    f32 = mybir.dt.float32
    nc = bacc.Bacc("TRN2", target_bir_lowering=False, debug=False,
                   num_devices=NCORES)

    io = {}

    def din(name, shape):
        io[name] = nc.dram_tensor(name, shape, f32, kind="ExternalInput")

    # host-relayouted inputs (pure transpose/reshape of the originals)
    din("x0h", [3, NPOS])          # x0h[c, t*128+gp*32+i] = pg[4t+gp, i, c]
    din("pgAh", [128, 96])         # [32gp+i, (t c)] = pg[4t+gp, i, c]
    din("pgSh", [4, NPTS * 32 * 3])  # [gp, (t m c)] = pg[4t+gp, m, c]
    din("pg_full", [1024, NPTS, 3])
    din("W1", [128, 3])
    din("W1Th", [3, 128])
    din("W2Th", [128, 2 * 128])    # blocks (kc=0, mo) of W2 [256,128]
    din("W3Th", [128, 16 * 128])   # blocks (kc, mo) of W3 [512,512]
    din("W4Th", [128, 12 * 128])   # blocks (kc, mo) of W4 [384,512]
    din("WfTh", [128, 18 * 128])   # blocks (kc, mo) of Wf [384,768]
    din("vecs", [128, 32])         # packed bias/affine columns
    din("b4row", [1, C])
    din("alpha_row", [1, C])
    io["out"] = nc.dram_tensor("out", [G, C], f32, kind="ExternalOutput")
    if debug:
        for nm, sh in [("dbg_f1h", [128, NPOS]), ("dbg_negkey", [128, 1024]),
                       ("dbg_A2", [128, 1024]), ("dbg_Kc", [128, 32]),
                       ("dbg_f3h0", [128, NPOS]), ("dbg_fg2", [128, C]),
                       ("dbg_U", [128, C]), ("dbg_mom", [128, 17]),
                       ("dbg_P", [128, C]), ("dbg_Q", [128, C]),
                       ("dbg_feat0", [128, C]), ("dbg_fg", [128, 256])]:
            io[nm] = nc.dram_tensor(nm, sh, f32, kind="ExternalOutput")

    with tile.TileContext(nc) as tc:
        _emit(nc, tc, tile, mybir, io, debug)
    nc.compile()
    return nc


def _emit(nc, tc, tile, mybir, io, debug):
    f32 = mybir.dt.float32
    f32r = mybir.dt.float32r
    Alu = mybir.AluOpType
    Act = mybir.ActivationFunctionType
    AX = mybir.AxisListType
    RG = [list(range(NCORES))]

    def mm(outap, lhsT, rhs, start, stop, rep=True):
        if rep:
            lhsT = lhsT.bitcast(f32r)
            rhs = rhs.bitcast(f32r)
        nc.tensor.matmul(outap, lhsT, rhs, start=start, stop=stop,
                         skip_group_check=True)

    ctx = ExitStack()
    per = ctx.enter_context(tc.tile_pool(name="per", bufs=1))
    ps_c = ctx.enter_context(tc.tile_pool(name="ps_c", bufs=1, space="PSUM"))
    dram = ctx.enter_context(tc.tile_pool(name="dram", bufs=1, space="DRAM"))

    # ---------------- constants ----------------
    ident = per.tile([128, 128], f32, name="ident")
    nc.gpsimd.memset(ident[:], 1.0)
    nc.gpsimd.affine_select(ident[:], ident[:], pattern=[[1, 128]],
                            compare_op=Alu.is_equal, fill=0.0, base=0,
                            channel_multiplier=-1)
    ones1x128 = per.tile([1, 128], f32, name="ones1x128")
    nc.gpsimd.memset(ones1x128[:], 1.0)
    nc.scalar.activation(ones1x128.bitcast(f32r), ones1x128[:], Act.Identity)
    ones128x1 = per.tile([128, 1], f32, name="ones128x1")
    nc.gpsimd.memset(ones128x1[:], 1.0)
    eps_col = per.tile([128, 1], f32, name="eps_col")
    nc.gpsimd.memset(eps_col[:], EPS)

    # ---------------- dummy collective (comm warm-up) ----------------
    warm_in = dram.tile([128, 1], f32, name="warm_in")
    warm_out = dram.tile([128, 1], f32, name="warm_out")
    nc.sync.dma_start(warm_in[:], ones128x1[:])
    nc.gpsimd.collective_compute("AllReduce", Alu.add, replica_groups=RG,
                                 ins=[warm_in.opt()], outs=[warm_out.opt()])

    # ---------------- load weights + vectors (host pre-laid-out) --------
    # x0 first on the SP queue: conv1 is the critical path
    conv_in_cm = tc.tile_pool(name="conv_in", bufs=1)
    conv_in = conv_in_cm.__enter__()
    x0 = conv_in.tile([3, NPOS], f32, name="x0")
    nc.sync.dma_start(x0[:], io["x0h"].ap())

    vecs = per.tile([128, 32], f32, name="vecs")
    nc.sync.dma_start(vecs[:], io["vecs"].ap())
    b1_sb = vecs[:, 0:1]
    gamma1_sb = vecs[:, 1:2]
    beta1_sb = vecs[:, 2:3]
    b2_sb = vecs[:, 3:5]
    b3_sb = vecs[:, 5:9]
    gamma3_sb = vecs[:, 9:13]
    beta3_sb = vecs[:, 13:17]
    b4_sb = vecs[:, 17:20]
    bf_sb = vecs[:, 20:23]
    gammaf_sb = vecs[:, 23:26]
    betaf_sb = vecs[:, 26:29]
    betaaff_sb = vecs[:, 29:32]
    W1_sb = per.tile([128, 3], f32, name="W1_sb")
    nc.sync.dma_start(W1_sb[:], io["W1"].ap())
    W1T = per.tile([3, 128], f32, name="W1T")
    nc.sync.dma_start(W1T[:], io["W1Th"].ap())
    b4row0 = per.tile([1, C], f32, name="b4row0")
    nc.sync.dma_start(b4row0[:], io["b4row"].ap())
    b4row = per.tile([1, C], f32, name="b4row")
    nc.scalar.activation(b4row.bitcast(f32r), b4row0[:], Act.Identity)
    alpha_row = per.tile([1, C], f32, name="alpha_row")
    nc.sync.dma_start(alpha_row[:], io["alpha_row"].ap())

    # bulk weight loads on the scalar queue so they don't block the SP queue.
    # f32r matmult inputs must come from a rounding instruction, not a DMA:
    # round W2T/W3T/W4T/b4row through the Act engine right after the loads
    # (WfT is only used in plain-f32 matmuls, conv1 runs in plain f32).
    wraw_cm = tc.tile_pool(name="wraw", bufs=1)
    wraw = wraw_cm.__enter__()
    wst = wraw.tile([128, 16 * 128], f32, name="wst")
    W2T = per.tile([128, 2 * 128], f32, name="W2T")
    nc.scalar.dma_start(wst[:, :2 * 128], io["W2Th"].ap())
    nc.scalar.activation(W2T.bitcast(f32r), wst[:, :2 * 128], Act.Identity)
    W3T = per.tile([128, 16 * 128], f32, name="W3T")
    nc.scalar.dma_start(wst[:], io["W3Th"].ap())
    nc.scalar.activation(W3T.bitcast(f32r), wst[:], Act.Identity)
    W4T = per.tile([128, 12 * 128], f32, name="W4T")
    nc.scalar.dma_start(wst[:, :12 * 128], io["W4Th"].ap())
    nc.scalar.activation(W4T.bitcast(f32r), wst[:, :12 * 128], Act.Identity)
    WfT = per.tile([128, 18 * 128], f32, name="WfT")
    nc.scalar.dma_start(WfT[:], io["WfTh"].ap())

    def wblk(wt, nr, kc, mo):
        return wt[:, (kc * nr + mo) * 128:(kc * nr + mo) * 128 + 128]

    def w4rhs(kc):  # pos-major rhs [128, 384] = blocks (kc, mo=0..2)
        return W4T[:, kc * 3 * 128:(kc * 3 + 3) * 128]

    def bn_scale_shift(var_ap, mu_ap, gam_ap, bet_ap, pref, n=1):
        std = per.tile([128, n], f32, name=pref + "_std")
        nc.scalar.activation(std[:], var_ap, Act.Sqrt, bias=eps_col[:])
        rstd = per.tile([128, n], f32, name=pref + "_rstd")
        nc.vector.reciprocal(rstd[:], std[:])
        sc = per.tile([128, n], f32, name=pref + "_sc")
        nc.vector.tensor_tensor(sc[:], rstd[:], gam_ap, op=Alu.mult)
        sh = per.tile([128, n], f32, name=pref + "_sh")
        nc.vector.tensor_tensor(sh[:], mu_ap, sc[:], op=Alu.mult)
        nc.vector.tensor_tensor(sh[:], bet_ap, sh[:], op=Alu.subtract)
        return sc, sh

    # ================ BN1 moments from global input ================
    bn1_cm = tc.tile_pool(name="bn1", bufs=1)
    bn1p = bn1_cm.__enter__()
    pgm = bn1p.tile([128, 768], f32, name="pgm")   # [128, (jj:8, i:32, c:3)]
    nc.gpsimd.dma_start(pgm[:], io["pg_full"].ap().rearrange(
        "(p jj) i c -> p (jj i c)", p=128).opt())
    mcols = bn1p.tile([128, 12], f32, name="mcols")
    pv = pgm.rearrange("p (j c) -> p j c", c=3)
    scr256 = bn1p.tile([128, 256], f32, name="scr256")
    for i in range(3):
        for j in range(3):
            nc.vector.scalar_tensor_tensor(
                scr256[:], pv[:, :, i], 1.0, pv[:, :, j],
                op0=Alu.mult, op1=Alu.mult,
                accum_out=mcols[:, 3 * i + j:3 * i + j + 1])
        nc.vector.tensor_reduce(mcols[:, 9 + i:10 + i], pv[:, :, i],
                                axis=AX.X, op=Alu.add)
    m12 = ps_c.tile([1, 12], f32, name="m12", tag="cps")
    mm(m12[:], ones128x1[:], mcols[:], True, True, rep=False)
    m12s = bn1p.tile([1, 12], f32, name="m12s")
    nc.scalar.activation(m12s[:], m12[:], Act.Identity, scale=1.0 / NTOT_POS)
    M2sb = bn1p.tile([3, 3], f32, name="M2sb")
    nc.gpsimd.dma_start(M2sb[:],
                        m12s[:1, :9].rearrange("1 (i j) -> 1 i j", i=3))
    mu3 = bn1p.tile([3, 1], f32, name="mu3")
    nc.gpsimd.dma_start(mu3[:], m12s[:1, 9:12])

    m1ps = ps_c.tile([128, 1], f32, name="m1ps", tag="cps")
    mm(m1ps[:], W1T[:], mu3[:], True, True, rep=False)   # W1 @ mu_p
    mvec = per.tile([128, 1], f32, name="mvec")
    nc.vector.tensor_copy(mvec[:], m1ps[:])
    wmps = ps_c.tile([128, 3], f32, name="wmps", tag="cps")
    mm(wmps[:], W1T[:], M2sb[:], True, True, rep=False)  # W1 @ M2
    # var of sc1*(W1 x + b1) is translation-invariant: var1 = E2raw - mvec^2
    # and the shift folds to sh1b = beta1 - sc1*mvec (b1 cancels)
    e2raw = per.tile([128, 1], f32, name="e2raw")
    scr3 = per.tile([128, 3], f32, name="scr3")
    nc.vector.scalar_tensor_tensor(scr3[:], wmps[:], 1.0, W1_sb[:],
                                   op0=Alu.mult, op1=Alu.mult,
                                   accum_out=e2raw[:])
    t_a = per.tile([128, 1], f32, name="t_a")
    nc.vector.tensor_tensor(t_a[:], mvec[:], mvec[:], op=Alu.mult)
    var1 = per.tile([128, 1], f32, name="var1")
    nc.vector.tensor_tensor(var1[:], e2raw[:], t_a[:], op=Alu.subtract)
    std1 = per.tile([128, 1], f32, name="std1")
    nc.scalar.activation(std1[:], var1[:], Act.Sqrt, bias=eps_col[:])
    rstd1 = per.tile([128, 1], f32, name="rstd1")
    nc.vector.reciprocal(rstd1[:], std1[:])
    sc1 = per.tile([128, 1], f32, name="sc1")
    nc.vector.tensor_tensor(sc1[:], rstd1[:], gamma1_sb[:], op=Alu.mult)
    sh1b = per.tile([128, 1], f32, name="sh1b")
    nc.vector.tensor_tensor(sh1b[:], mvec[:], sc1[:], op=Alu.mult)
    nc.vector.tensor_tensor(sh1b[:], beta1_sb[:], sh1b[:], op=Alu.subtract)
    bn1_cm.__exit__(None, None, None)

    # early DMAs for the selection phase (consumed later)
    selin_cm = tc.tile_pool(name="selin", bufs=1)
    selin = selin_cm.__enter__()
    pgA = selin.tile([128, 96], f32, name="pgA")
    nc.gpsimd.dma_start(pgA[:], io["pgAh"].ap())
    pgB = selin.tile([128, 3072], f32, name="pgB")
    for gp in range(4):
        nc.gpsimd.dma_start(
            pgB[32 * gp:32 * gp + 32],
            io["pgSh"].ap()[gp:gp + 1].broadcast_to([32, 3072]))

    # wbias = Wf[:, C:] @ beta_aff + bf   (channel-major [128, 3])
    wbias_ps = ps_c.tile([128, 3], f32, name="wbias_ps", tag="cps")
    for mo in range(3):
        for kc in range(3):
            mm(wbias_ps[:, mo:mo + 1], wblk(WfT, 3, 3 + kc, mo),
               betaaff_sb[:, kc:kc + 1], kc == 0, kc == 2, rep=False)
    wbias = per.tile([128, 3], f32, name="wbias")
    nc.vector.tensor_tensor(wbias[:], wbias_ps[:], bf_sb[:], op=Alu.add)


    # ---------------- data-independent H-phase constants ----------------
    # onesblk[32gp+n, gp'] = 1 iff gp' == gp; stationary for per-t U writes
    onesblk = per.tile([128, 4], f32, name="onesblk")
    nc.gpsimd.memset(onesblk[:], 1.0)
    nc.gpsimd.affine_select(onesblk[:], onesblk[:], pattern=[[-32, 4]],
                            compare_op=Alu.is_ge, fill=0.0, base=0,
                            channel_multiplier=1)
    nc.gpsimd.affine_select(onesblk[:], onesblk[:], pattern=[[32, 4]],
                            compare_op=Alu.is_ge, fill=0.0, base=31,
                            channel_multiplier=-1)
    # alpha replicated to all partitions
    alphar_ps = ps_c.tile([128, C], f32, name="alphar_ps", tag="cps")
    alpha_row = per.tile([1, C], f32, name="alpha_row")
    nc.sync.dma_start(alpha_row[:], io["alpha_row"].ap())
    mm(alphar_ps[:], ones1x128[:], alpha_row[:], True, True, rep=False)
    alphar = per.tile([128, C], f32, name="alphar")
    nc.scalar.activation(alphar[:], alphar_ps[:], Act.Identity)

    # ================ conv1 / conv2 ================
    ps_mm_cm = tc.tile_pool(name="ps_mm", bufs=6, space="PSUM")
    ps_mm = ps_mm_cm.__enter__()
    act3_cm = tc.tile_pool(name="act3", bufs=1)
    act3 = act3_cm.__enter__()
    act1_cm = tc.tile_pool(name="act1", bufs=1)
    act1 = act1_cm.__enter__()

    f1h = act1.tile([128, NPOS], f32, name="f1h")
    for nt in range(8):
        ps = ps_mm.tile([128, 512], f32, name="mmps", tag="mmps")
        mm(ps[:], W1T[:], x0[:, nt * 512:(nt + 1) * 512], True, True,
           rep=False)
        nc.scalar.activation(f1h[:, nt * 512:(nt + 1) * 512].bitcast(f32r),
                             ps[:], Act.Relu, bias=sh1b[:], scale=sc1[:])
    if debug:
        nc.sync.dma_start(io["dbg_f1h"].ap(), f1h[:])

    fg = per.tile([128, 256], f32, name="fg")  # [128, (mo:2, g:128)]
    f2 = [act3.tile([128, NPOS], f32, name=f"f2_{mo}") for mo in range(2)]
    for mo in range(2):
        for nt in range(8):
            ps = ps_mm.tile([128, 512], f32, name="mmps", tag="mmps")
            mm(ps[:], wblk(W2T, 2, 0, mo), f1h[:, nt * 512:(nt + 1) * 512],
               True, True)
            nc.scalar.activation(
                f2[mo][:, nt * 512:(nt + 1) * 512].bitcast(f32r), ps[:],
                Act.Identity, bias=b2_sb[:, mo:mo + 1])
        # per-group max as soon as f2[mo] is complete (feeds conv3 kc=mo)
        nc.vector.tensor_reduce(fg[:, mo * 128:(mo + 1) * 128]
                                .bitcast(f32r),
                                f2[mo].rearrange("p (g i) -> p g i", i=32),
                                axis=AX.X, op=Alu.max)
    act1_cm.__exit__(None, None, None)
    if debug:
        nc.sync.dma_start(io["dbg_fg"].ap(), fg[:])

    # ================ conv3 (stats in, bias copy out) ================
    f3 = [per.tile([128, NPOS], f32, name=f"f3_{mo}") for mo in range(4)]
    stats3 = per.tile([128, 4 * 8 * 6], f32, name="stats3")
    mv3 = per.tile([128, 8], f32, name="mv3")
    for mo in range(4):
        for ntc in range(4):
            pss = [ps_mm.tile([128, 512], f32, name="mmps", tag="mmps")
                   for _ in range(2)]
            # f2 blocks first so the fg reduce is off the critical path
            for kc in (2, 3, 0, 1):
                for j, nt in enumerate((2 * ntc, 2 * ntc + 1)):
                    if kc < 2:
                        rhs = fg[:, kc * 128 + nt * 16:
                                 kc * 128 + (nt + 1) * 16] \
                            .unsqueeze(2).broadcast_to([128, 16, 32])
                    else:
                        rhs = f2[kc - 2][:, nt * 512:(nt + 1) * 512]
                    mm(pss[j][:], wblk(W3T, 4, kc, mo), rhs, kc == 2,
                       kc == 1)
            for j, nt in enumerate((2 * ntc, 2 * ntc + 1)):
                dst = f3[mo][:, nt * 512:(nt + 1) * 512].bitcast(f32r)
                nc.scalar.activation(dst, pss[j][:], Act.Identity,
                                     bias=b3_sb[:, mo:mo + 1])
                nc.vector.bn_stats(
                    stats3[:, (mo * 8 + nt) * 6:(mo * 8 + nt) * 6 + 6],
                    dst)
        nc.vector.bn_aggr(mv3[:, mo * 2:mo * 2 + 2],
                          stats3[:, mo * 48:(mo + 1) * 48])
    act3_cm.__exit__(None, None, None)
    ps_mm_cm.__exit__(None, None, None)

    # local (sum, sumsq) per channel -> AllReduce (launch ASAP)
    # psum stats lack +b3, but b3 cancels in the variance; ship raw
    # sums/sumsq and add b3 to the global mean after the AllReduce
    bnloc = per.tile([128, 8], f32, name="bnloc")
    mv3v = mv3.rearrange("p (m two) -> p two m", two=2)
    bnlv = bnloc.rearrange("p (m two) -> p two m", two=2)
    nc.scalar.activation(bnlv[:, 0, :], mv3v[:, 0, :], Act.Identity,
                         scale=float(NPOS))
    scrb3 = per.tile([128, 4], f32, name="scrb3")
    nc.vector.scalar_tensor_tensor(scrb3[:], mv3v[:, 0, :], 1.0,
                                   mv3v[:, 0, :], op0=Alu.mult, op1=Alu.mult)
    nc.vector.tensor_tensor(scrb3[:], scrb3[:], mv3v[:, 1, :], op=Alu.add)
    nc.scalar.activation(bnlv[:, 1, :], scrb3[:], Act.Identity,
                         scale=float(NPOS))
    cc3_in = dram.tile([128, 8], f32, name="cc3_in")
    cc3_out = dram.tile([128, 8], f32, name="cc3_out")
    nc.sync.dma_start(cc3_in[:], bnloc[:])
    nc.gpsimd.collective_compute("AllReduce", Alu.add, replica_groups=RG,
                                 ins=[cc3_in.opt()], outs=[cc3_out.opt()])

    # ====== selection, declared here so it runs inside the BN3 window ======
    hconst_cm = tc.tile_pool(name="hconst", bufs=1)
    hc = hconst_cm.__enter__()
    # W_B zero background (block-diag A2T copied in below)
    W_B = hc.tile([128, NPOS], f32, name="W_B")
    nc.gpsimd.memset(W_B[:], 0.0)
    nc.gpsimd.tensor_copy(W_B.bitcast(f32r), W_B[:])
    # onesU[32*gp+n, t*128 + m] = 1 iff m == 4t+gp
    onesU = hc.tile([128, 32 * 128], f32, name="onesU")
    nc.gpsimd.memset(onesU[:], 0.0)
    nc.gpsimd.tensor_copy(onesU.bitcast(f32r), onesU[:])
    for t in range(32):
        nc.gpsimd.tensor_copy(
            onesU[:, t * 128 + 4 * t:t * 128 + 4 * t + 4].bitcast(f32r),
            onesblk[:])
    sel_b = tc.tile_pool(name="sel_b", bufs=1)
    sb = sel_b.__enter__()
    # negkey[32gp+n, t*32+m] = sum_c (pgA[.,t,c] - 0.5*pgB_c)*pgB_c
    scr1 = sb.tile([128, 1024], f32, name="scr1")
    negkey = sb.tile([128, 1024], f32, name="negkey")
    for cdim in range(3):
        pgB_c = pgB.rearrange("p (t m c) -> p t m c", t=32, m=32)[:, :, :, cdim]
        pgA_c = pgA.rearrange("p (t c) -> p t c", c=3)[:, :, cdim] \
            .unsqueeze(2).broadcast_to([128, 32, 32])
        dst = scr1[:] if cdim > 0 else negkey[:]
        dstv = dst.rearrange("p (t m) -> p t m", t=32)
        nc.vector.scalar_tensor_tensor(dstv, pgB_c, -0.5, pgA_c,
                                       op0=Alu.mult, op1=Alu.add)
        nc.vector.tensor_tensor(dstv, dstv, pgB_c, op=Alu.mult)
        if cdim > 0:
            nc.vector.tensor_tensor(negkey[:], negkey[:], scr1[:],
                                    op=Alu.add)

    top8 = sb.tile([128, 8], f32, name="top8")
    repl = sb.tile([128, 1024], f32, name="repl", tag="repl")
    for t in range(32):
        nc.vector.max(top8[:], negkey[:, t * 32:(t + 1) * 32])
        nc.vector.match_replace(repl[:, t * 32:(t + 1) * 32], top8[:],
                                negkey[:, t * 32:(t + 1) * 32], BIG_NEG)
    A2 = sb.tile([128, 1024], f32, name="A2")
    nc.vector.tensor_scalar(A2[:], repl[:], BIG_NEG, None, op0=Alu.is_equal)
    if debug:
        nc.sync.dma_start(io["dbg_negkey"].ap(), negkey[:])
        nc.sync.dma_start(io["dbg_A2"].ap(), A2[:])

    A2T = sb.tile([128, 1024], f32, name="A2T", tag="repl")
    nc.vector.transpose(A2T[:], A2[:])
    # Kc[32gp+m, t] = K + sum_n A[n, m];  Kw = Kc - 2K (U weights)
    Kc = per.tile([128, 32], f32, name="Kc")
    nc.vector.tensor_reduce(Kc[:],
                            A2T.rearrange("p (t n) -> p t n", t=32),
                            axis=AX.X, op=Alu.add)
    nc.vector.tensor_scalar(Kc[:], Kc[:], float(K), None, op0=Alu.add)
    Kw = per.tile([128, 32], f32, name="Kw")
    nc.vector.tensor_scalar(Kw[:], Kc[:], -2.0 * K, None, op0=Alu.add)
    if debug:
        nc.sync.dma_start(io["dbg_Kc"].ap(), Kc[:])

    # W_B[32gp+m, t*128+32gp+n] = A2T[32gp+m, t*32+n]  (block-diag lhsT)
    for gp in range(4):
        nc.vector.tensor_copy(
            W_B[32 * gp:32 * gp + 32].rearrange(
                "p (t q) -> p t q", t=32)[:, :, 32 * gp:32 * gp + 32]
            .bitcast(f32r),
            A2T[32 * gp:32 * gp + 32].rearrange("p (t n) -> p t n", t=32))
    sel_b.__exit__(None, None, None)

    # ====== BN3 post-collective scale/shift + chunked ReLU3 ======
    g3 = per.tile([128, 8], f32, name="g3")
    nc.sync.dma_start(g3[:], cc3_out[:])
    gmu3 = per.tile([128, 4], f32, name="gmu3")
    nc.scalar.activation(gmu3[:], g3.rearrange("p (m two) -> p two m",
                                               two=2)[:, 0, :],
                         Act.Identity, scale=1.0 / NTOT_POS)
    ge23 = per.tile([128, 4], f32, name="ge23")
    nc.scalar.activation(ge23[:], g3.rearrange("p (m two) -> p two m",
                                               two=2)[:, 1, :],
                         Act.Identity, scale=1.0 / NTOT_POS)
    gvar3 = per.tile([128, 4], f32, name="gvar3")
    nc.vector.tensor_tensor(gvar3[:], gmu3[:], gmu3[:], op=Alu.mult)
    nc.vector.tensor_tensor(gvar3[:], ge23[:], gvar3[:], op=Alu.subtract)
    sc3, sh3 = bn_scale_shift(gvar3[:], gmu3[:], gamma3_sb[:], beta3_sb[:],
                              "bn3", n=4)

    # ReLU3 chunk nt covers H iterations t in [4nt, 4nt+4); interleave the
    # chunks into the H loop so the in-order Act queue doesn't drain all of
    # ReLU3 before feat t=0
    def relu3_chunk(nt, eng="pool"):
        for mo in range(4):
            sl = f3[mo][:, nt * 512:(nt + 1) * 512]
            if eng == "act":
                nc.scalar.activation(sl.bitcast(f32r), sl, Act.Relu,
                                     bias=sh3[:, mo:mo + 1],
                                     scale=sc3[:, mo:mo + 1])
            else:
                nc.gpsimd.tensor_scalar(sl.bitcast(f32r), sl,
                                        sc3[:, mo:mo + 1], sh3[:, mo:mo + 1],
                                        op0=Alu.mult, op1=Alu.add)
                nc.gpsimd.tensor_scalar(sl.bitcast(f32r), sl.bitcast(f32r),
                                        0.0, None, op0=Alu.max)

    # ================ H phase: conv4 pos-major, U, moments, fg2 =========
    fg2 = per.tile([128, C], f32, name="fg2")     # [128ch, (mo:3, g:128)]
    fg2v = fg2.rearrange("p (mo g) -> p mo g", mo=3)
    sqcol = per.tile([128, 32], f32, name="sqcol")  # ||feat_pos||^2 per t
    acc2 = per.tile([128, 32], f32, name="acc2")    # feat . h per t
    scrSq = per.tile([128, C], f32, name="scrSq")
    scrH = per.tile([128, C], f32, name="scrH")
    U_sb = per.tile([128, C], f32, name="U_sb")
    t1col = per.tile([128, 1], f32, name="t1col")

    with tc.tile_pool(name="psU", bufs=1, space="PSUM") as psU:
        Ups = psU.tile([128, C], f32, name="Ups", tag="hold")
        with tc.tile_pool(name="featpool", bufs=4) as featpool, \
             tc.tile_pool(name="psF", bufs=3, space="PSUM") as psF, \
             tc.tile_pool(name="psT", bufs=2, space="PSUM") as psT:
            relu3_chunk(0, eng="act")
            relu3_chunk(1)
            for t in range(32):
                fps = psF.tile([128, C], f32, name="fps", tag="fps")
                for kc in range(4):
                    mm(fps[:], f3[kc][:, t * 128:(t + 1) * 128], w4rhs(kc),
                       kc == 0, False)
                mm(fps[:], ones1x128[:], b4row[:], False, True)  # + b4
                feat = featpool.tile([128, C], f32, name="feat", tag="feat")
                nc.scalar.activation(feat.bitcast(f32r), fps[:], Act.Identity)
                if debug and t == 0:
                    nc.sync.dma_start(io["dbg_feat0"].ap(), feat[:])
                # ||feat||^2 per position (Act engine, reads PSUM directly)
                nc.scalar.activation(scrSq[:], fps[:], Act.Square,
                                     accum_out=sqcol[:, t:t + 1])
                # fg2 via PE transpose (f32r) + combined per-group max
                pst = psT.tile([128, C], f32, name="tps", tag="tps")
                for mo in range(3):
                    nc.tensor.matmul(
                        pst[:, mo * 128:(mo + 1) * 128],
                        feat[:, mo * 128:(mo + 1) * 128],
                        ident[:], is_transpose=True,
                        skip_group_check=True)
                nc.vector.tensor_reduce(
                    fg2v[:, :, 4 * t:4 * t + 4],
                    pst.rearrange("p (mo gp i) -> p mo gp i", mo=3, i=32),
                    axis=AX.X, op=Alu.max)
                # neighbor sum (for the T2 moment only)
                hps = psF.tile([128, C], f32, name="hps", tag="hps",
                               bufs=1)
                mm(hps[:], W_B[:, t * 128:(t + 1) * 128], feat[:],
                   True, True)
                nc.vector.scalar_tensor_tensor(
                    scrH[:], feat[:], 1.0, hps[:],
                    op0=Alu.mult, op1=Alu.mult, accum_out=acc2[:, t:t + 1])
                # U path: wfeat = (Kc - 2K) * feat
                wfeat = featpool.tile([128, C], f32, name="wfeat", tag="wf")
                nc.vector.tensor_scalar(wfeat.bitcast(f32r), feat[:],
                                        Kw[:, t:t + 1], None, op0=Alu.mult)
                mm(Ups[:], onesU[:, t * 128:(t + 1) * 128], wfeat[:],
                   t == 0, t == 31)
                if t % 4 == 0 and t // 4 + 2 < 8:
                    nt = t // 4 + 2
                    relu3_chunk(nt, eng="act" if nt % 2 == 1 else "pool")
        nc.scalar.activation(U_sb[:], Ups[:], Act.Identity,
                             accum_out=t1col[:])
    hconst_cm.__exit__(None, None, None)
    selin_cm.__exit__(None, None, None)
    wraw_cm.__exit__(None, None, None)
    conv_in_cm.__exit__(None, None, None)
    if debug:
        nc.sync.dma_start(io["dbg_fg2"].ap(), fg2[:])

    # t2col = sum_t (Kc*sq) - 2*sum_t acc2
    a1r = per.tile([128, 1], f32, name="a1r")
    scr32 = per.tile([128, 32], f32, name="scr32")
    nc.vector.scalar_tensor_tensor(scr32[:], sqcol[:], 1.0, Kc[:],
                                   op0=Alu.mult, op1=Alu.mult,
                                   accum_out=a1r[:])
    a2r = per.tile([128, 1], f32, name="a2r")
    nc.vector.tensor_reduce(a2r[:], acc2[:], axis=AX.X, op=Alu.add)
    t2col = per.tile([128, 1], f32, name="t2col")
    nc.vector.scalar_tensor_tensor(t2col[:], a2r[:], -2.0, a1r[:],
                                   op0=Alu.mult, op1=Alu.add)

    # V = alpha * U / (n*K)  (group-major), then transpose to channel-major
    V_sb = per.tile([128, C], f32, name="V_sb")
    nc.vector.scalar_tensor_tensor(V_sb[:], U_sb[:], 1.0 / (NPTS * K),
                                   alphar[:], op0=Alu.mult, op1=Alu.mult)
    Vc = per.tile([128, C], f32, name="Vc")
    for mo in range(3):
        pstv = ps_c.tile([128, 128], f32, name="wtps", tag="cps")
        nc.tensor.transpose(pstv[:], V_sb[:, mo * 128:(mo + 1) * 128],
                            ident[:])
        nc.vector.tensor_copy(Vc[:, mo * 128:(mo + 1) * 128], pstv[:])
    if debug:
        nc.sync.dma_start(io["dbg_U"].ap(), U_sb[:])

    # ================ P/Q matmuls + moments ================
    P_sb = per.tile([128, C], f32, name="P_sb")
    Q_sb = per.tile([128, C], f32, name="Q_sb")
    mom = per.tile([128, 17], f32, name="mom")
    scrP = per.tile([128, 128], f32, name="scrP")
    with tc.tile_pool(name="psQ", bufs=1, space="PSUM") as psQ:
        Pps = psQ.tile([128, C], f32, name="Pps", tag="holdP")
        Qps = psQ.tile([128, C], f32, name="Qps", tag="holdQ")
        for mo in range(3):
            for kc in range(3):
                mm(Pps[:, mo * 128:(mo + 1) * 128], wblk(WfT, 3, kc, mo),
                   fg2[:, kc * 128:(kc + 1) * 128], kc == 0, kc == 2,
                   rep=False)
                mm(Qps[:, mo * 128:(mo + 1) * 128], wblk(WfT, 3, 3 + kc, mo),
                   Vc[:, kc * 128:(kc + 1) * 128], kc == 0, kc == 2,
                   rep=False)
        scrQ = per.tile([128, 128], f32, name="scrQ")
        scrPQ = per.tile([128, 128], f32, name="scrPQ")
        for mo in range(3):
            nc.scalar.activation(P_sb[:, mo * 128:(mo + 1) * 128],
                                 Pps[:, mo * 128:(mo + 1) * 128],
                                 Act.Identity, bias=wbias[:, mo:mo + 1],
                                 accum_out=mom[:, mo:mo + 1])
            nc.scalar.activation(Q_sb[:, mo * 128:(mo + 1) * 128],
                                 Qps[:, mo * 128:(mo + 1) * 128], Act.Identity,
                                 accum_out=mom[:, 3 + mo:4 + mo])
            nc.scalar.activation(scrP[:], P_sb[:, mo * 128:(mo + 1) * 128],
                                 Act.Square, accum_out=mom[:, 6 + mo:7 + mo])
            nc.vector.scalar_tensor_tensor(
                scrQ[:], Q_sb[:, mo * 128:(mo + 1) * 128], 1.0,
                Q_sb[:, mo * 128:(mo + 1) * 128], op0=Alu.mult, op1=Alu.mult,
                accum_out=mom[:, 9 + mo:10 + mo])
            nc.vector.scalar_tensor_tensor(
                scrPQ[:], P_sb[:, mo * 128:(mo + 1) * 128], 1.0,
                Q_sb[:, mo * 128:(mo + 1) * 128], op0=Alu.mult, op1=Alu.mult,
                accum_out=mom[:, 12 + mo:13 + mo])
    nc.vector.tensor_copy(mom[:, 15:16], t1col[:])
    nc.vector.tensor_copy(mom[:, 16:17], t2col[:])
    if debug:
        nc.sync.dma_start(io["dbg_P"].ap(), P_sb[:])
        nc.sync.dma_start(io["dbg_Q"].ap(), Q_sb[:])
        nc.sync.dma_start(io["dbg_mom"].ap(), mom[:])

    ccf_in = dram.tile([128, 17], f32, name="ccf_in")
    ccf_out = dram.tile([128, 17], f32, name="ccf_out")
    nc.sync.dma_start(ccf_in[:], mom[:])
    nc.gpsimd.collective_compute("AllReduce", Alu.add, replica_groups=RG,
                                 ins=[ccf_in.opt()], outs=[ccf_out.opt()])
    gmom = per.tile([128, 17], f32, name="gmom")
    nc.sync.dma_start(gmom[:], ccf_out[:])

    # T1/T2: partition-sum via ones matmul, broadcast back via K=1 matmul
    t12_ps = ps_c.tile([1, 2], f32, name="t12_ps", tag="cps")
    mm(t12_ps[:], ones128x1[:], gmom[:, 15:17], True, True, rep=False)
    t12 = per.tile([1, 2], f32, name="t12")
    nc.vector.tensor_copy(t12[:], t12_ps[:])
    t12b_ps = ps_c.tile([128, 2], f32, name="t12b_ps", tag="cps")
    mm(t12b_ps[:], ones1x128[:], t12[:], True, True, rep=False)
    T1 = per.tile([128, 1], f32, name="T1")
    nc.vector.tensor_copy(T1[:], t12b_ps[:, 0:1])
    T2 = per.tile([128, 1], f32, name="T2")
    nc.vector.tensor_copy(T2[:], t12b_ps[:, 1:2])

    # s = 1 / (std + EPS); var = (T2 - T1^2/N) / (N-1)
    tA = per.tile([128, 1], f32, name="tA")
    nc.vector.tensor_tensor(tA[:], T1[:], T1[:], op=Alu.mult)
    tB = per.tile([128, 1], f32, name="tB")
    nc.vector.scalar_tensor_tensor(tB[:], tA[:], -1.0 / NTOT_DX, T2[:],
                                   op0=Alu.mult, op1=Alu.add)
    stdx = per.tile([128, 1], f32, name="stdx")
    nc.scalar.activation(stdx[:], tB[:], Act.Sqrt,
                         scale=1.0 / (NTOT_DX - 1))
    nc.vector.tensor_scalar(stdx[:], stdx[:], EPS, None, op0=Alu.add)
    s_col = per.tile([128, 1], f32, name="s_col")
    nc.vector.reciprocal(s_col[:], stdx[:])
    s2_col = per.tile([128, 1], f32, name="s2_col")
    nc.vector.tensor_tensor(s2_col[:], s_col[:], s_col[:], op=Alu.mult)
    ts2 = per.tile([128, 1], f32, name="ts2")
    nc.vector.tensor_scalar(ts2[:], s_col[:], 2.0, None, op0=Alu.mult)

    # ================ BNf + output (vectorized over mo) ================
    # muf = (sumP + s*sumQ) / NB
    muf = per.tile([128, 3], f32, name="muf")
    nc.vector.scalar_tensor_tensor(muf[:], gmom[:, 3:6], s_col[:],
                                   gmom[:, 0:3], op0=Alu.mult, op1=Alu.add)
    nc.scalar.activation(muf[:], muf[:], Act.Identity, scale=1.0 / NB)
    # e2f = (sumP2 + 2s*sumPQ + s^2*sumQ2) / NB
    e2f = per.tile([128, 3], f32, name="e2f")
    nc.vector.scalar_tensor_tensor(e2f[:], gmom[:, 12:15], ts2[:],
                                   gmom[:, 6:9], op0=Alu.mult, op1=Alu.add)
    nc.vector.scalar_tensor_tensor(e2f[:], gmom[:, 9:12], s2_col[:],
                                   e2f[:], op0=Alu.mult, op1=Alu.add)
    nc.scalar.activation(e2f[:], e2f[:], Act.Identity, scale=1.0 / NB)
    varf = per.tile([128, 3], f32, name="varf")
    nc.vector.tensor_tensor(varf[:], muf[:], muf[:], op=Alu.mult)
    nc.vector.tensor_tensor(varf[:], e2f[:], varf[:], op=Alu.subtract)
    scf, shf = bn_scale_shift(varf[:], muf[:], gammaf_sb[:], betaf_sb[:],
                              "bnf", n=3)
    outsb = per.tile([128, C], f32, name="outsb")
    zf = per.tile([128, C], f32, name="zf")
    nc.vector.scalar_tensor_tensor(zf[:], Q_sb[:], s_col[:], P_sb[:],
                                   op0=Alu.mult, op1=Alu.add)
    for mo in range(3):
        fused = per.tile([128, 128], f32, name=f"fused_{mo}")
        nc.scalar.activation(fused[:], zf[:, mo * 128:(mo + 1) * 128],
                             Act.Relu, bias=shf[:, mo:mo + 1],
                             scale=scf[:, mo:mo + 1])
        pst = ps_c.tile([128, 128], f32, name="wtps", tag="cps")
        nc.tensor.transpose(pst[:], fused[:], ident[:])
        nc.vector.tensor_copy(outsb[:, mo * 128:(mo + 1) * 128], pst[:])

    nc.sync.dma_start(io["out"].ap(), outsb[:])
    ctx.close()


def _get_built():
    global _BUILT
    if _BUILT is None:
        _BUILT = _build(DEBUG)
    return _BUILT


def _wt_host(W, rows, cols):
    """Host layout matching wblk(): out[q, (kc*nr+mo)*128+p] =
    W[mo*128+p, kc*128+q]."""
    nr, kg = rows // 128, cols // 128
    blk = W.reshape(nr, 128, kg, 128)          # [mo, p, kc, q]
    return np.ascontiguousarray(
        blk.transpose(3, 2, 0, 1).reshape(128, kg * nr * 128))


def _vec_host(v):
    n = v.shape[0]
    nt = n // 128
    return np.ascontiguousarray(v.reshape(nt, 128).T)


def make_in_maps(inputs):
    pgf = np.ascontiguousarray(
        np.asarray(inputs["point_groups"], dtype=np.float32).reshape(
            1024, NPTS, 3))
    f = lambda n: np.asarray(inputs[n], dtype=np.float32)
    base = {
        "pg_full": pgf,
        "W1": np.ascontiguousarray(f("W1")),
        "W1Th": np.ascontiguousarray(f("W1").T),
        "W2Th": _wt_host(f("W2"), 256, 128),
        "W3Th": _wt_host(f("W3"), 512, 512),
        "W4Th": _wt_host(f("W4"), C, 512),
        "WfTh": _wt_host(f("Wf"), C, 2 * C),
        "vecs": np.ascontiguousarray(np.concatenate(
            [_vec_host(f("b1")), _vec_host(f("gamma1")),
             _vec_host(f("beta1")), _vec_host(f("b2")),
             _vec_host(f("b3")), _vec_host(f("gamma3")),
             _vec_host(f("beta3")), _vec_host(f("b4")),
             _vec_host(f("bf")), _vec_host(f("gammaf")),
             _vec_host(f("betaf")),
             _vec_host(f("beta_aff").reshape(C))], axis=1)),
        "b4row": np.ascontiguousarray(f("b4").reshape(1, C)),
        "alpha_row": np.ascontiguousarray(f("alpha").reshape(1, C)),
    }
    in_maps = []
    for c in range(NCORES):
        m = dict(base)
        pg = pgf[c * G:(c + 1) * G]            # [(t gp), i, c]
        pgr = pg.reshape(32, 4, NPTS, 3)       # [t, gp, i, c]
        m["x0h"] = np.ascontiguousarray(
            pgr.transpose(3, 0, 1, 2).reshape(3, NPOS))
        m["pgAh"] = np.ascontiguousarray(
            pgr.transpose(1, 2, 0, 3).reshape(128, 96))
        m["pgSh"] = np.ascontiguousarray(
            pgr.transpose(1, 0, 2, 3).reshape(4, NPTS * 32 * 3))
        in_maps.append(m)
    return in_maps


def kernel(**inputs):
    from concourse.bass_utils import run_bass_kernel_spmd

    nc = _get_built()
    in_maps = make_in_maps(inputs)
    res = run_bass_kernel_spmd(nc, in_maps, list(range(NCORES)))
    full = np.concatenate([res.results[c]["out"] for c in range(NCORES)],
                          axis=0)
    return full.reshape(4, 256, C)


# revision 60
# speedup vs baseline: 1.0340x; 1.0340x over previous
"""Trainium2 Bass kernel for nn_EnhancedEncoder (gnn_message_passing).

Data-parallel over the 1024 flattened groups: 128 groups per core on 8 cores.
All intermediates stay in SBUF. The KNN gather is reformulated with counts:
U[g] = sum_m (cnt_m - K) feat_m needs only per-position selection counts, so
the H loop has no neighbor-sum matmul on its critical chain; the neighbor sum
hps is kept only for the global T2 moment. fg2 comes from per-tile PE
transposes of position-major feat (conv4 is computed once). Weight/layout
transposes are done host-side in make_in_maps (pure relayout, no arithmetic).
Cross-core reductions: warm-up AllReduce + BN3 stats + one merged final round
(global std of dx + fusion BN stats); the KNN selection phase is declared
between the BN3 collective launch and its consumers so the in-order engine
queues execute it inside the collective's latency window.

Position indexing per core: pos = t*128 + gp*32 + i  (t in [0,32), gp in
[0,4), i in [0,32)); group id g = 4*t + gp.  Channel-major tensors are
[ch_tile(128), pos(4096)]; feat tiles are [4*32 points, 384]; per-group
vectors are [*, g] with g = 4*t + gp.
"""
import sys
from contextlib import ExitStack

for _p in ("/opt/trn_rl_repo",):
    if _p not in sys.path:
        sys.path.insert(0, _p)

import numpy as np

NCORES = 8
G = 128            # groups per core
NPTS = 32          # points per group
NPOS = G * NPTS    # 4096 positions per core
C = 384            # encoder channels
K = 8              # knn group size
EPS = 1e-5
NTOT_POS = 1024 * NPTS          # global positions (BN1/BN3 denominator)
NTOT_DX = 1024 * NPTS * K * C   # global dx element count (std denominator)
NB = 1024                       # global batch of groups (BNf denominator)
BIG_NEG = -1e30
DEBUG = False

_BUILT = None


def _build(debug=False):
    import concourse.bacc as bacc
    import concourse.tile as tile
    from concourse import mybir

    f32 = mybir.dt.float32
    nc = bacc.Bacc("TRN2", target_bir_lowering=False, debug=False,
                   num_devices=NCORES)

    io = {}

    def din(name, shape):
        io[name] = nc.dram_tensor(name, shape, f32, kind="ExternalInput")

    # host-relayouted inputs (pure transpose/reshape of the originals)
    din("x0h", [3, NPOS])          # x0h[c, t*128+gp*32+i] = pg[4t+gp, i, c]
    din("pgAh", [128, 96])         # [32gp+i, (t c)] = pg[4t+gp, i, c]
    din("pgSh", [4, NPTS * 32 * 3])  # [gp, (t m c)] = pg[4t+gp, m, c]
    din("pg_full", [1024, NPTS, 3])
    din("W1", [128, 3])
    din("W1Th", [3, 128])
    din("W2Th", [128, 2 * 128])    # blocks (kc=0, mo) of W2 [256,128]
    din("W3Th", [128, 16 * 128])   # blocks (kc, mo) of W3 [512,512]
    din("W4Th", [128, 12 * 128])   # blocks (kc, mo) of W4 [384,512]
    din("WfTh", [128, 18 * 128])   # blocks (kc, mo) of Wf [384,768]
    din("vecs", [128, 32])         # packed bias/affine columns
    din("b4row", [1, C])
    din("alpha_row", [1, C])
    io["out"] = nc.dram_tensor("out", [G, C], f32, kind="ExternalOutput")
    if debug:
        for nm, sh in [("dbg_f1h", [128, NPOS]), ("dbg_negkey", [128, 1024]),
                       ("dbg_A2", [128, 1024]), ("dbg_Kc", [128, 32]),
                       ("dbg_f3h0", [128, NPOS]), ("dbg_fg2", [128, C]),
                       ("dbg_U", [128, C]), ("dbg_mom", [128, 17]),
                       ("dbg_P", [128, C]), ("dbg_Q", [128, C]),
                       ("dbg_feat0", [128, C]), ("dbg_fg", [128, 256])]:
            io[nm] = nc.dram_tensor(nm, sh, f32, kind="ExternalOutput")

    with tile.TileContext(nc) as tc:
        _emit(nc, tc, tile, mybir, io, debug)
    nc.compile()
    return nc


def _emit(nc, tc, tile, mybir, io, debug):
    f32 = mybir.dt.float32
    f32r = mybir.dt.float32r
    Alu = mybir.AluOpType
    Act = mybir.ActivationFunctionType
    AX = mybir.AxisListType
    RG = [list(range(NCORES))]

    def mm(outap, lhsT, rhs, start, stop, rep=True):
        if rep:
            lhsT = lhsT.bitcast(f32r)
            rhs = rhs.bitcast(f32r)
        nc.tensor.matmul(outap, lhsT, rhs, start=start, stop=stop,
                         skip_group_check=True)

    ctx = ExitStack()
    per = ctx.enter_context(tc.tile_pool(name="per", bufs=1))
    ps_c = ctx.enter_context(tc.tile_pool(name="ps_c", bufs=1, space="PSUM"))
    dram = ctx.enter_context(tc.tile_pool(name="dram", bufs=1, space="DRAM"))

    # ---------------- constants ----------------
    ident = per.tile([128, 128], f32, name="ident")
    nc.gpsimd.memset(ident[:], 1.0)
    nc.gpsimd.affine_select(ident[:], ident[:], pattern=[[1, 128]],
                            compare_op=Alu.is_equal, fill=0.0, base=0,
                            channel_multiplier=-1)
    ones1x128 = per.tile([1, 128], f32, name="ones1x128")
    nc.gpsimd.memset(ones1x128[:], 1.0)
    nc.scalar.activation(ones1x128.bitcast(f32r), ones1x128[:], Act.Identity)
    ones128x1 = per.tile([128, 1], f32, name="ones128x1")
    nc.gpsimd.memset(ones128x1[:], 1.0)
    eps_col = per.tile([128, 1], f32, name="eps_col")
    nc.gpsimd.memset(eps_col[:], EPS)

    # ---------------- dummy collective (comm warm-up) ----------------
    warm_in = dram.tile([128, 1], f32, name="warm_in")
    warm_out = dram.tile([128, 1], f32, name="warm_out")
    nc.sync.dma_start(warm_in[:], ones128x1[:])
    nc.gpsimd.collective_compute("AllReduce", Alu.add, replica_groups=RG,
                                 ins=[warm_in.opt()], outs=[warm_out.opt()])

    # ---------------- load weights + vectors (host pre-laid-out) --------
    # x0 first on the SP queue: conv1 is the critical path
    conv_in_cm = tc.tile_pool(name="conv_in", bufs=1)
    conv_in = conv_in_cm.__enter__()
    x0 = conv_in.tile([3, NPOS], f32, name="x0")
    nc.sync.dma_start(x0[:], io["x0h"].ap())

    vecs = per.tile([128, 32], f32, name="vecs")
    nc.sync.dma_start(vecs[:], io["vecs"].ap())
    b1_sb = vecs[:, 0:1]
    gamma1_sb = vecs[:, 1:2]
    beta1_sb = vecs[:, 2:3]
    b2_sb = vecs[:, 3:5]
    b3_sb = vecs[:, 5:9]
    gamma3_sb = vecs[:, 9:13]
    beta3_sb = vecs[:, 13:17]
    b4_sb = vecs[:, 17:20]
    bf_sb = vecs[:, 20:23]
    gammaf_sb = vecs[:, 23:26]
    betaf_sb = vecs[:, 26:29]
    betaaff_sb = vecs[:, 29:32]
    W1_sb = per.tile([128, 3], f32, name="W1_sb")
    nc.sync.dma_start(W1_sb[:], io["W1"].ap())
    W1T = per.tile([3, 128], f32, name="W1T")
    nc.sync.dma_start(W1T[:], io["W1Th"].ap())
    b4row0 = per.tile([1, C], f32, name="b4row0")
    nc.sync.dma_start(b4row0[:], io["b4row"].ap())
    b4row = per.tile([1, C], f32, name="b4row")
    nc.scalar.activation(b4row.bitcast(f32r), b4row0[:], Act.Identity)
    alpha_row = per.tile([1, C], f32, name="alpha_row")
    nc.sync.dma_start(alpha_row[:], io["alpha_row"].ap())

    # bulk weight loads on the scalar queue so they don't block the SP queue.
    # f32r matmult inputs must come from a rounding instruction, not a DMA:
    # round W2T/W3T/W4T/b4row through the Act engine right after the loads
    # (WfT is only used in plain-f32 matmuls, conv1 runs in plain f32).
    wraw_cm = tc.tile_pool(name="wraw", bufs=1)
    wraw = wraw_cm.__enter__()
    wst = wraw.tile([128, 18 * 128], f32, name="wst")
    W2T = per.tile([128, 2 * 128], f32, name="W2T")
    nc.scalar.dma_start(wst[:, 16 * 128:18 * 128], io["W2Th"].ap())
    nc.scalar.activation(W2T.bitcast(f32r), wst[:, 16 * 128:18 * 128],
                         Act.Identity)
    W3T = per.tile([128, 16 * 128], f32, name="W3T")
    nc.scalar.dma_start(wst[:, :16 * 128], io["W3Th"].ap())
    nc.scalar.activation(W3T.bitcast(f32r), wst[:, :16 * 128], Act.Identity)
    W4T = per.tile([128, 12 * 128], f32, name="W4T")
    nc.scalar.dma_start(wst[:, :12 * 128], io["W4Th"].ap())
    nc.scalar.activation(W4T.bitcast(f32r), wst[:, :12 * 128], Act.Identity)
    WfT = per.tile([128, 18 * 128], f32, name="WfT")
    nc.scalar.dma_start(WfT[:], io["WfTh"].ap())

    def wblk(wt, nr, kc, mo):
        return wt[:, (kc * nr + mo) * 128:(kc * nr + mo) * 128 + 128]

    def w4rhs(kc):  # pos-major rhs [128, 384] = blocks (kc, mo=0..2)
        return W4T[:, kc * 3 * 128:(kc * 3 + 3) * 128]

    def bn_scale_shift(var_ap, mu_ap, gam_ap, bet_ap, pref, n=1):
        std = per.tile([128, n], f32, name=pref + "_std")
        nc.scalar.activation(std[:], var_ap, Act.Sqrt, bias=eps_col[:])
        rstd = per.tile([128, n], f32, name=pref + "_rstd")
        nc.vector.reciprocal(rstd[:], std[:])
        sc = per.tile([128, n], f32, name=pref + "_sc")
        nc.vector.tensor_tensor(sc[:], rstd[:], gam_ap, op=Alu.mult)
        sh = per.tile([128, n], f32, name=pref + "_sh")
        nc.vector.tensor_tensor(sh[:], mu_ap, sc[:], op=Alu.mult)
        nc.vector.tensor_tensor(sh[:], bet_ap, sh[:], op=Alu.subtract)
        return sc, sh

    # ================ BN1 moments from global input ================
    bn1_cm = tc.tile_pool(name="bn1", bufs=1)
    bn1p = bn1_cm.__enter__()
    pgm = bn1p.tile([128, 768], f32, name="pgm")   # [128, (jj:8, i:32, c:3)]
    nc.gpsimd.dma_start(pgm[:], io["pg_full"].ap().rearrange(
        "(p jj) i c -> p (jj i c)", p=128).opt())
    mcols = bn1p.tile([128, 12], f32, name="mcols")
    pv = pgm.rearrange("p (j c) -> p j c", c=3)
    scr256 = bn1p.tile([128, 256], f32, name="scr256")
    for i in range(3):
        for j in range(3):
            nc.vector.scalar_tensor_tensor(
                scr256[:], pv[:, :, i], 1.0, pv[:, :, j],
                op0=Alu.mult, op1=Alu.mult,
                accum_out=mcols[:, 3 * i + j:3 * i + j + 1])
        nc.vector.tensor_reduce(mcols[:, 9 + i:10 + i], pv[:, :, i],
                                axis=AX.X, op=Alu.add)
    m12 = ps_c.tile([1, 12], f32, name="m12", tag="cps")
    mm(m12[:], ones128x1[:], mcols[:], True, True, rep=False)
    m12s = bn1p.tile([1, 12], f32, name="m12s")
    nc.scalar.activation(m12s[:], m12[:], Act.Identity, scale=1.0 / NTOT_POS)
    M2sb = bn1p.tile([3, 3], f32, name="M2sb")
    nc.gpsimd.dma_start(M2sb[:],
                        m12s[:1, :9].rearrange("1 (i j) -> 1 i j", i=3))
    mu3 = bn1p.tile([3, 1], f32, name="mu3")
    nc.gpsimd.dma_start(mu3[:], m12s[:1, 9:12])

    m1ps = ps_c.tile([128, 1], f32, name="m1ps", tag="cps")
    mm(m1ps[:], W1T[:], mu3[:], True, True, rep=False)   # W1 @ mu_p
    mvec = per.tile([128, 1], f32, name="mvec")
    nc.vector.tensor_copy(mvec[:], m1ps[:])
    wmps = ps_c.tile([128, 3], f32, name="wmps", tag="cps")
    mm(wmps[:], W1T[:], M2sb[:], True, True, rep=False)  # W1 @ M2
    # var of sc1*(W1 x + b1) is translation-invariant: var1 = E2raw - mvec^2
    # and the shift folds to sh1b = beta1 - sc1*mvec (b1 cancels)
    e2raw = per.tile([128, 1], f32, name="e2raw")
    scr3 = per.tile([128, 3], f32, name="scr3")
    nc.vector.scalar_tensor_tensor(scr3[:], wmps[:], 1.0, W1_sb[:],
                                   op0=Alu.mult, op1=Alu.mult,
                                   accum_out=e2raw[:])
    t_a = per.tile([128, 1], f32, name="t_a")
    nc.vector.tensor_tensor(t_a[:], mvec[:], mvec[:], op=Alu.mult)
    var1 = per.tile([128, 1], f32, name="var1")
    nc.vector.tensor_tensor(var1[:], e2raw[:], t_a[:], op=Alu.subtract)
    std1 = per.tile([128, 1], f32, name="std1")
    nc.scalar.activation(std1[:], var1[:], Act.Sqrt, bias=eps_col[:])
    rstd1 = per.tile([128, 1], f32, name="rstd1")
    nc.vector.reciprocal(rstd1[:], std1[:])
    sc1 = per.tile([128, 1], f32, name="sc1")
    nc.vector.tensor_tensor(sc1[:], rstd1[:], gamma1_sb[:], op=Alu.mult)
    sh1b = per.tile([128, 1], f32, name="sh1b")
    nc.vector.tensor_tensor(sh1b[:], mvec[:], sc1[:], op=Alu.mult)
    nc.vector.tensor_tensor(sh1b[:], beta1_sb[:], sh1b[:], op=Alu.subtract)
    bn1_cm.__exit__(None, None, None)

    # early DMAs for the selection phase (consumed later)
    selin_cm = tc.tile_pool(name="selin", bufs=1)
    selin = selin_cm.__enter__()
    pgA = selin.tile([128, 96], f32, name="pgA")
    nc.gpsimd.dma_start(pgA[:], io["pgAh"].ap())
    pgB = selin.tile([128, 3072], f32, name="pgB")
    for gp in range(4):
        nc.gpsimd.dma_start(
            pgB[32 * gp:32 * gp + 32],
            io["pgSh"].ap()[gp:gp + 1].broadcast_to([32, 3072]))

    # wbias = Wf[:, C:] @ beta_aff + bf   (channel-major [128, 3])
    wbias_ps = ps_c.tile([128, 3], f32, name="wbias_ps", tag="cps")
    for mo in range(3):
        for kc in range(3):
            mm(wbias_ps[:, mo:mo + 1], wblk(WfT, 3, 3 + kc, mo),
               betaaff_sb[:, kc:kc + 1], kc == 0, kc == 2, rep=False)
    wbias = per.tile([128, 3], f32, name="wbias")
    nc.vector.tensor_tensor(wbias[:], wbias_ps[:], bf_sb[:], op=Alu.add)


    # ---------------- data-independent H-phase constants ----------------
    # onesblk[32gp+n, gp'] = 1 iff gp' == gp; stationary for per-t U writes
    onesblk = per.tile([128, 4], f32, name="onesblk")
    nc.gpsimd.memset(onesblk[:], 1.0)
    nc.gpsimd.affine_select(onesblk[:], onesblk[:], pattern=[[-32, 4]],
                            compare_op=Alu.is_ge, fill=0.0, base=0,
                            channel_multiplier=1)
    nc.gpsimd.affine_select(onesblk[:], onesblk[:], pattern=[[32, 4]],
                            compare_op=Alu.is_ge, fill=0.0, base=31,
                            channel_multiplier=-1)
    # alpha replicated to all partitions
    alphar_ps = ps_c.tile([128, C], f32, name="alphar_ps", tag="cps")
    alpha_row = per.tile([1, C], f32, name="alpha_row")
    nc.sync.dma_start(alpha_row[:], io["alpha_row"].ap())
    mm(alphar_ps[:], ones1x128[:], alpha_row[:], True, True, rep=False)
    alphar = per.tile([128, C], f32, name="alphar")
    nc.scalar.activation(alphar[:], alphar_ps[:], Act.Identity)

    # ================ conv1 / conv2 ================
    ps_mm_cm = tc.tile_pool(name="ps_mm", bufs=6, space="PSUM")
    ps_mm = ps_mm_cm.__enter__()
    act3_cm = tc.tile_pool(name="act3", bufs=1)
    act3 = act3_cm.__enter__()
    act1_cm = tc.tile_pool(name="act1", bufs=1)
    act1 = act1_cm.__enter__()

    f1h = act1.tile([128, NPOS], f32, name="f1h")
    for nt in range(8):
        ps = ps_mm.tile([128, 512], f32, name="mmps", tag="mmps")
        mm(ps[:], W1T[:], x0[:, nt * 512:(nt + 1) * 512], True, True,
           rep=False)
        nc.scalar.activation(f1h[:, nt * 512:(nt + 1) * 512].bitcast(f32r),
                             ps[:], Act.Relu, bias=sh1b[:], scale=sc1[:])
    if debug:
        nc.sync.dma_start(io["dbg_f1h"].ap(), f1h[:])

    fg = per.tile([128, 256], f32, name="fg")  # [128, (mo:2, g:128)]
    f2 = [act3.tile([128, NPOS], f32, name=f"f2_{mo}") for mo in range(2)]
    for mo in range(2):
        for nt in range(8):
            ps = ps_mm.tile([128, 512], f32, name="mmps", tag="mmps")
            mm(ps[:], wblk(W2T, 2, 0, mo), f1h[:, nt * 512:(nt + 1) * 512],
               True, True)
            nc.scalar.activation(
                f2[mo][:, nt * 512:(nt + 1) * 512].bitcast(f32r), ps[:],
                Act.Identity, bias=b2_sb[:, mo:mo + 1])
            # per-group max per chunk (16 groups) so fg is ready for conv3
            nc.vector.tensor_reduce(
                fg[:, mo * 128 + nt * 16:mo * 128 + (nt + 1) * 16]
                .bitcast(f32r),
                f2[mo][:, nt * 512:(nt + 1) * 512]
                .rearrange("p (g i) -> p g i", i=32),
                axis=AX.X, op=Alu.max)
    act1_cm.__exit__(None, None, None)
    if debug:
        nc.sync.dma_start(io["dbg_fg"].ap(), fg[:])

    # ================ conv3 (stats in, bias copy out) ================
    f3 = [per.tile([128, NPOS], f32, name=f"f3_{mo}") for mo in range(4)]
    stats3 = per.tile([128, 4 * 8 * 6], f32, name="stats3")
    mv3 = per.tile([128, 8], f32, name="mv3")
    for mo in range(4):
        for ntc in range(4):
            pss = [ps_mm.tile([128, 512], f32, name="mmps", tag="mmps")
                   for _ in range(2)]
            # f2 blocks first so the fg reduce is off the critical path
            for kc in (2, 3, 0, 1):
                for j, nt in enumerate((2 * ntc, 2 * ntc + 1)):
                    if kc < 2:
                        rhs = fg[:, kc * 128 + nt * 16:
                                 kc * 128 + (nt + 1) * 16] \
                            .unsqueeze(2).broadcast_to([128, 16, 32])
                    else:
                        rhs = f2[kc - 2][:, nt * 512:(nt + 1) * 512]
                    mm(pss[j][:], wblk(W3T, 4, kc, mo), rhs, kc == 2,
                       kc == 1)
            for j, nt in enumerate((2 * ntc, 2 * ntc + 1)):
                dst = f3[mo][:, nt * 512:(nt + 1) * 512].bitcast(f32r)
                nc.scalar.activation(dst, pss[j][:], Act.Identity,
                                     bias=b3_sb[:, mo:mo + 1])
                nc.vector.bn_stats(
                    stats3[:, (mo * 8 + nt) * 6:(mo * 8 + nt) * 6 + 6],
                    dst)
        nc.vector.bn_aggr(mv3[:, mo * 2:mo * 2 + 2],
                          stats3[:, mo * 48:(mo + 1) * 48])
    act3_cm.__exit__(None, None, None)
    ps_mm_cm.__exit__(None, None, None)

    # local (sum, sumsq) per channel -> AllReduce (launch ASAP)
    # psum stats lack +b3, but b3 cancels in the variance; ship raw
    # sums/sumsq and add b3 to the global mean after the AllReduce
    bnloc = per.tile([128, 8], f32, name="bnloc")
    mv3v = mv3.rearrange("p (m two) -> p two m", two=2)
    bnlv = bnloc.rearrange("p (m two) -> p two m", two=2)
    nc.vector.tensor_scalar(bnlv[:, 0, :], mv3v[:, 0, :], float(NPOS), None,
                            op0=Alu.mult)
    scrb3 = per.tile([128, 4], f32, name="scrb3")
    nc.vector.scalar_tensor_tensor(scrb3[:], mv3v[:, 0, :], 1.0,
                                   mv3v[:, 0, :], op0=Alu.mult, op1=Alu.mult)
    nc.vector.tensor_tensor(scrb3[:], scrb3[:], mv3v[:, 1, :], op=Alu.add)
    nc.vector.tensor_scalar(bnlv[:, 1, :], scrb3[:], float(NPOS), None,
                            op0=Alu.mult)
    cc3_in = dram.tile([128, 8], f32, name="cc3_in")
    cc3_out = dram.tile([128, 8], f32, name="cc3_out")
    nc.sync.dma_start(cc3_in[:], bnloc[:])
    nc.gpsimd.collective_compute("AllReduce", Alu.add, replica_groups=RG,
                                 ins=[cc3_in.opt()], outs=[cc3_out.opt()])

    # ====== selection, declared here so it runs inside the BN3 window ======
    hconst_cm = tc.tile_pool(name="hconst", bufs=1)
    hc = hconst_cm.__enter__()
    # W_B zero background (block-diag A2T copied in below)
    W_B = hc.tile([128, NPOS], f32, name="W_B")
    nc.gpsimd.memset(W_B[:], 0.0)
    nc.gpsimd.tensor_copy(W_B.bitcast(f32r), W_B[:])
    # onesU[32*gp+n, t*128 + m] = 1 iff m == 4t+gp
    onesU = hc.tile([128, 32 * 128], f32, name="onesU")
    nc.gpsimd.memset(onesU[:], 0.0)
    nc.gpsimd.tensor_copy(onesU.bitcast(f32r), onesU[:])
    for t in range(32):
        nc.gpsimd.tensor_copy(
            onesU[:, t * 128 + 4 * t:t * 128 + 4 * t + 4].bitcast(f32r),
            onesblk[:])
    sel_b = tc.tile_pool(name="sel_b", bufs=1)
    sb = sel_b.__enter__()
    # negkey[32gp+n, t*32+m] = sum_c (pgA[.,t,c] - 0.5*pgB_c)*pgB_c
    scr1 = sb.tile([128, 1024], f32, name="scr1")
    negkey = sb.tile([128, 1024], f32, name="negkey")
    for cdim in range(3):
        pgB_c = pgB.rearrange("p (t m c) -> p t m c", t=32, m=32)[:, :, :, cdim]
        pgA_c = pgA.rearrange("p (t c) -> p t c", c=3)[:, :, cdim] \
            .unsqueeze(2).broadcast_to([128, 32, 32])
        dst = scr1[:] if cdim > 0 else negkey[:]
        dstv = dst.rearrange("p (t m) -> p t m", t=32)
        nc.vector.scalar_tensor_tensor(dstv, pgB_c, -0.5, pgA_c,
                                       op0=Alu.mult, op1=Alu.add)
        nc.vector.tensor_tensor(dstv, dstv, pgB_c, op=Alu.mult)
        if cdim > 0:
            nc.vector.tensor_tensor(negkey[:], negkey[:], scr1[:],
                                    op=Alu.add)

    top8 = sb.tile([128, 8], f32, name="top8")
    repl = sb.tile([128, 1024], f32, name="repl", tag="repl")
    for t in range(32):
        nc.vector.max(top8[:], negkey[:, t * 32:(t + 1) * 32])
        nc.vector.match_replace(repl[:, t * 32:(t + 1) * 32], top8[:],
                                negkey[:, t * 32:(t + 1) * 32], BIG_NEG)
    A2 = sb.tile([128, 1024], f32, name="A2")
    nc.vector.tensor_scalar(A2[:], repl[:], BIG_NEG, None, op0=Alu.is_equal)
    if debug:
        nc.sync.dma_start(io["dbg_negkey"].ap(), negkey[:])
        nc.sync.dma_start(io["dbg_A2"].ap(), A2[:])

    A2T = sb.tile([128, 1024], f32, name="A2T", tag="repl")
    nc.vector.transpose(A2T[:], A2[:])
    # Kc[32gp+m, t] = K + sum_n A[n, m];  Kw = Kc - 2K (U weights)
    Kc = per.tile([128, 32], f32, name="Kc")
    nc.vector.tensor_reduce(Kc[:],
                            A2T.rearrange("p (t n) -> p t n", t=32),
                            axis=AX.X, op=Alu.add)
    nc.vector.tensor_scalar(Kc[:], Kc[:], float(K), None, op0=Alu.add)
    Kw = per.tile([128, 32], f32, name="Kw")
    nc.vector.tensor_scalar(Kw[:], Kc[:], -2.0 * K, None, op0=Alu.add)
    if debug:
        nc.sync.dma_start(io["dbg_Kc"].ap(), Kc[:])

    # W_B[32gp+m, t*128+32gp+n] = A2T[32gp+m, t*32+n]  (block-diag lhsT)
    for gp in range(4):
        nc.vector.tensor_copy(
            W_B[32 * gp:32 * gp + 32].rearrange(
                "p (t q) -> p t q", t=32)[:, :, 32 * gp:32 * gp + 32]
            .bitcast(f32r),
            A2T[32 * gp:32 * gp + 32].rearrange("p (t n) -> p t n", t=32))
    sel_b.__exit__(None, None, None)

    # ====== BN3 post-collective scale/shift + chunked ReLU3 ======
    g3 = per.tile([128, 8], f32, name="g3")
    nc.sync.dma_start(g3[:], cc3_out[:])
    gmu3 = per.tile([128, 4], f32, name="gmu3")
    nc.scalar.activation(gmu3[:], g3.rearrange("p (m two) -> p two m",
                                               two=2)[:, 0, :],
                         Act.Identity, scale=1.0 / NTOT_POS)
    ge23 = per.tile([128, 4], f32, name="ge23")
    nc.scalar.activation(ge23[:], g3.rearrange("p (m two) -> p two m",
                                               two=2)[:, 1, :],
                         Act.Identity, scale=1.0 / NTOT_POS)
    gvar3 = per.tile([128, 4], f32, name="gvar3")
    nc.vector.tensor_tensor(gvar3[:], gmu3[:], gmu3[:], op=Alu.mult)
    nc.vector.tensor_tensor(gvar3[:], ge23[:], gvar3[:], op=Alu.subtract)
    sc3, sh3 = bn_scale_shift(gvar3[:], gmu3[:], gamma3_sb[:], beta3_sb[:],
                              "bn3", n=4)

    # ReLU3 chunk nt covers H iterations t in [4nt, 4nt+4); interleave the
    # chunks into the H loop so the in-order Act queue doesn't drain all of
    # ReLU3 before feat t=0
    def relu3_chunk(nt, eng="pool"):
        for mo in range(4):
            sl = f3[mo][:, nt * 512:(nt + 1) * 512]
            if eng == "act":
                nc.scalar.activation(sl.bitcast(f32r), sl, Act.Relu,
                                     bias=sh3[:, mo:mo + 1],
                                     scale=sc3[:, mo:mo + 1])
            else:
                nc.gpsimd.tensor_scalar(sl.bitcast(f32r), sl,
                                        sc3[:, mo:mo + 1], sh3[:, mo:mo + 1],
                                        op0=Alu.mult, op1=Alu.add)
                nc.gpsimd.tensor_scalar(sl.bitcast(f32r), sl.bitcast(f32r),
                                        0.0, None, op0=Alu.max)

    # ================ H phase: conv4 pos-major, U, moments, fg2 =========
    fg2 = per.tile([128, C], f32, name="fg2")     # [128ch, (mo:3, g:128)]
    fg2v = fg2.rearrange("p (mo g) -> p mo g", mo=3)
    sqcol = per.tile([128, 32], f32, name="sqcol")  # ||feat_pos||^2 per t
    acc2 = per.tile([128, 32], f32, name="acc2")    # feat . h per t
    scrSq = per.tile([128, C], f32, name="scrSq")
    scrH = per.tile([128, C], f32, name="scrH")
    U_sb = per.tile([128, C], f32, name="U_sb")
    t1col = per.tile([128, 1], f32, name="t1col")

    with tc.tile_pool(name="psU", bufs=1, space="PSUM") as psU:
        Ups = psU.tile([128, C], f32, name="Ups", tag="hold")
        with tc.tile_pool(name="featpool", bufs=4) as featpool, \
             tc.tile_pool(name="psF", bufs=3, space="PSUM") as psF, \
             tc.tile_pool(name="psT", bufs=2, space="PSUM") as psT:
            for mo in range(2):
                sl = f3[mo][:, 0:512]
                nc.scalar.activation(sl.bitcast(f32r), sl, Act.Relu,
                                     bias=sh3[:, mo:mo + 1],
                                     scale=sc3[:, mo:mo + 1])
            for mo in range(2, 4):
                sl = f3[mo][:, 0:512]
                nc.vector.tensor_scalar(sl.bitcast(f32r), sl,
                                        sc3[:, mo:mo + 1], sh3[:, mo:mo + 1],
                                        op0=Alu.mult, op1=Alu.add)
                nc.vector.tensor_scalar(sl.bitcast(f32r), sl.bitcast(f32r),
                                        0.0, None, op0=Alu.max)
            relu3_chunk(1)
            for t in range(32):
                fps = psF.tile([128, C], f32, name="fps", tag="fps")
                for kc in range(4):
                    mm(fps[:], f3[kc][:, t * 128:(t + 1) * 128], w4rhs(kc),
                       kc == 0, False)
                mm(fps[:], ones1x128[:], b4row[:], False, True)  # + b4
                feat = featpool.tile([128, C], f32, name="feat", tag="feat")
                nc.scalar.activation(feat.bitcast(f32r), fps[:], Act.Identity)
                if debug and t == 0:
                    nc.sync.dma_start(io["dbg_feat0"].ap(), feat[:])
                # ||feat||^2 per position (from SBUF so fps frees earlier)
                nc.scalar.activation(scrSq[:], feat[:], Act.Square,
                                     accum_out=sqcol[:, t:t + 1])
                # fg2 via PE transpose (f32r) + combined per-group max
                pst = psT.tile([128, C], f32, name="tps", tag="tps")
                for mo in range(3):
                    nc.tensor.matmul(
                        pst[:, mo * 128:(mo + 1) * 128],
                        feat[:, mo * 128:(mo + 1) * 128],
                        ident[:], is_transpose=True,
                        skip_group_check=True)
                nc.vector.tensor_reduce(
                    fg2v[:, :, 4 * t:4 * t + 4],
                    pst.rearrange("p (mo gp i) -> p mo gp i", mo=3, i=32),
                    axis=AX.X, op=Alu.max)
                # neighbor sum (for the T2 moment only)
                hps = psF.tile([128, C], f32, name="hps", tag="hps",
                               bufs=1)
                mm(hps[:], W_B[:, t * 128:(t + 1) * 128], feat[:],
                   True, True)
                nc.vector.scalar_tensor_tensor(
                    scrH[:], feat[:], 1.0, hps[:],
                    op0=Alu.mult, op1=Alu.mult, accum_out=acc2[:, t:t + 1])
                # U path: wfeat = (Kc - 2K) * feat
                wfeat = featpool.tile([128, C], f32, name="wfeat", tag="wf")
                nc.vector.tensor_scalar(wfeat.bitcast(f32r), feat[:],
                                        Kw[:, t:t + 1], None, op0=Alu.mult)
                mm(Ups[:], onesU[:, t * 128:(t + 1) * 128], wfeat[:],
                   t == 0, t == 31)
                if t % 4 == 0 and t // 4 + 2 < 8:
                    nt = t // 4 + 2
                    relu3_chunk(nt, eng="act" if nt % 2 == 1 else "pool")
        nc.scalar.activation(U_sb[:], Ups[:], Act.Identity,
                             accum_out=t1col[:])
    hconst_cm.__exit__(None, None, None)
    selin_cm.__exit__(None, None, None)
    wraw_cm.__exit__(None, None, None)
    conv_in_cm.__exit__(None, None, None)
    if debug:
        nc.sync.dma_start(io["dbg_fg2"].ap(), fg2[:])

    # t2col = sum_t (Kc*sq) - 2*sum_t acc2
    a1r = per.tile([128, 1], f32, name="a1r")
    scr32 = per.tile([128, 32], f32, name="scr32")
    nc.vector.scalar_tensor_tensor(scr32[:], sqcol[:], 1.0, Kc[:],
                                   op0=Alu.mult, op1=Alu.mult,
                                   accum_out=a1r[:])
    a2r = per.tile([128, 1], f32, name="a2r")
    nc.vector.tensor_reduce(a2r[:], acc2[:], axis=AX.X, op=Alu.add)
    t2col = per.tile([128, 1], f32, name="t2col")
    nc.vector.scalar_tensor_tensor(t2col[:], a2r[:], -2.0, a1r[:],
                                   op0=Alu.mult, op1=Alu.add)

    # V = alpha * U / (n*K)  (group-major), then transpose to channel-major
    V_sb = per.tile([128, C], f32, name="V_sb")
    nc.vector.scalar_tensor_tensor(V_sb[:], U_sb[:], 1.0 / (NPTS * K),
                                   alphar[:], op0=Alu.mult, op1=Alu.mult)
    Vc = per.tile([128, C], f32, name="Vc")
    for mo in range(3):
        pstv = ps_c.tile([128, 128], f32, name="wtps", tag="cps")
        nc.tensor.transpose(pstv[:], V_sb[:, mo * 128:(mo + 1) * 128],
                            ident[:])
        nc.vector.tensor_copy(Vc[:, mo * 128:(mo + 1) * 128], pstv[:])
    if debug:
        nc.sync.dma_start(io["dbg_U"].ap(), U_sb[:])

    # ================ P/Q matmuls + moments ================
    P_sb = per.tile([128, C], f32, name="P_sb")
    Q_sb = per.tile([128, C], f32, name="Q_sb")
    mom = per.tile([128, 17], f32, name="mom")
    scrP = per.tile([128, 128], f32, name="scrP")
    with tc.tile_pool(name="psQ", bufs=1, space="PSUM") as psQ:
        Pps = psQ.tile([128, C], f32, name="Pps", tag="holdP")
        Qps = psQ.tile([128, C], f32, name="Qps", tag="holdQ")
        for mo in range(3):
            for kc in range(3):
                mm(Pps[:, mo * 128:(mo + 1) * 128], wblk(WfT, 3, kc, mo),
                   fg2[:, kc * 128:(kc + 1) * 128], kc == 0, kc == 2,
                   rep=False)
        for mo in range(3):
            for kc in range(3):
                mm(Qps[:, mo * 128:(mo + 1) * 128], wblk(WfT, 3, 3 + kc, mo),
                   Vc[:, kc * 128:(kc + 1) * 128], kc == 0, kc == 2,
                   rep=False)
        scrQ = per.tile([128, 128], f32, name="scrQ")
        scrPQ = per.tile([128, 128], f32, name="scrPQ")
        for mo in range(3):
            nc.scalar.activation(P_sb[:, mo * 128:(mo + 1) * 128],
                                 Pps[:, mo * 128:(mo + 1) * 128],
                                 Act.Identity, bias=wbias[:, mo:mo + 1],
                                 accum_out=mom[:, mo:mo + 1])
            nc.scalar.activation(Q_sb[:, mo * 128:(mo + 1) * 128],
                                 Qps[:, mo * 128:(mo + 1) * 128], Act.Identity,
                                 accum_out=mom[:, 3 + mo:4 + mo])
            nc.scalar.activation(scrP[:], P_sb[:, mo * 128:(mo + 1) * 128],
                                 Act.Square, accum_out=mom[:, 6 + mo:7 + mo])
            nc.vector.scalar_tensor_tensor(
                scrQ[:], Q_sb[:, mo * 128:(mo + 1) * 128], 1.0,
                Q_sb[:, mo * 128:(mo + 1) * 128], op0=Alu.mult, op1=Alu.mult,
                accum_out=mom[:, 9 + mo:10 + mo])
            nc.vector.scalar_tensor_tensor(
                scrPQ[:], P_sb[:, mo * 128:(mo + 1) * 128], 1.0,
                Q_sb[:, mo * 128:(mo + 1) * 128], op0=Alu.mult, op1=Alu.mult,
                accum_out=mom[:, 12 + mo:13 + mo])
    # partition-sum t1/t2 and broadcast before the AllReduce so gmom holds
    # the global scalars directly on every partition
    t12l = per.tile([128, 2], f32, name="t12l")
    nc.vector.tensor_copy(t12l[:, 0:1], t1col[:])
    nc.vector.tensor_copy(t12l[:, 1:2], t2col[:])
    t12_ps = ps_c.tile([1, 2], f32, name="t12_ps", tag="cps")
    mm(t12_ps[:], ones128x1[:], t12l[:], True, True, rep=False)
    t12 = per.tile([1, 2], f32, name="t12")
    nc.vector.tensor_copy(t12[:], t12_ps[:])
    t12b_ps = ps_c.tile([128, 2], f32, name="t12b_ps", tag="cps")
    mm(t12b_ps[:], ones1x128[:], t12[:], True, True, rep=False)
    nc.vector.tensor_copy(mom[:, 15:17], t12b_ps[:])
    if debug:
        nc.sync.dma_start(io["dbg_P"].ap(), P_sb[:])
        nc.sync.dma_start(io["dbg_Q"].ap(), Q_sb[:])
        nc.sync.dma_start(io["dbg_mom"].ap(), mom[:])

    ccf_in = dram.tile([128, 17], f32, name="ccf_in")
    ccf_out = dram.tile([128, 17], f32, name="ccf_out")
    nc.sync.dma_start(ccf_in[:], mom[:])
    nc.gpsimd.collective_compute("AllReduce", Alu.add, replica_groups=RG,
                                 ins=[ccf_in.opt()], outs=[ccf_out.opt()])
    gmom = per.tile([128, 17], f32, name="gmom")
    nc.sync.dma_start(gmom[:], ccf_out[:])

    T1 = gmom[:, 15:16]
    T2 = gmom[:, 16:17]

    # s = 1 / (std + EPS); var = (T2 - T1^2/N) / (N-1)
    tA = per.tile([128, 1], f32, name="tA")
    nc.vector.tensor_tensor(tA[:], T1, T1, op=Alu.mult)
    tB = per.tile([128, 1], f32, name="tB")
    nc.vector.scalar_tensor_tensor(tB[:], tA[:], -1.0 / NTOT_DX, T2,
                                   op0=Alu.mult, op1=Alu.add)
    stdx = per.tile([128, 1], f32, name="stdx")
    nc.scalar.activation(stdx[:], tB[:], Act.Sqrt,
                         scale=1.0 / (NTOT_DX - 1))
    nc.vector.tensor_scalar(stdx[:], stdx[:], EPS, None, op0=Alu.add)
    s_col = per.tile([128, 1], f32, name="s_col")
    nc.vector.reciprocal(s_col[:], stdx[:])
    s2_col = per.tile([128, 1], f32, name="s2_col")
    nc.vector.tensor_tensor(s2_col[:], s_col[:], s_col[:], op=Alu.mult)
    ts2 = per.tile([128, 1], f32, name="ts2")
    nc.vector.tensor_scalar(ts2[:], s_col[:], 2.0, None, op0=Alu.mult)

    # ================ BNf + output (vectorized over mo) ================
    # muf = (sumP + s*sumQ) / NB
    muf = per.tile([128, 3], f32, name="muf")
    nc.vector.scalar_tensor_tensor(muf[:], gmom[:, 3:6], s_col[:],
                                   gmom[:, 0:3], op0=Alu.mult, op1=Alu.add)
    nc.scalar.activation(muf[:], muf[:], Act.Identity, scale=1.0 / NB)
    # e2f = (sumP2 + 2s*sumPQ + s^2*sumQ2) / NB
    e2f = per.tile([128, 3], f32, name="e2f")
    nc.vector.scalar_tensor_tensor(e2f[:], gmom[:, 12:15], ts2[:],
                                   gmom[:, 6:9], op0=Alu.mult, op1=Alu.add)
    nc.vector.scalar_tensor_tensor(e2f[:], gmom[:, 9:12], s2_col[:],
                                   e2f[:], op0=Alu.mult, op1=Alu.add)
    nc.scalar.activation(e2f[:], e2f[:], Act.Identity, scale=1.0 / NB)
    varf = per.tile([128, 3], f32, name="varf")
    nc.vector.tensor_tensor(varf[:], muf[:], muf[:], op=Alu.mult)
    nc.vector.tensor_tensor(varf[:], e2f[:], varf[:], op=Alu.subtract)
    scf, shf = bn_scale_shift(varf[:], muf[:], gammaf_sb[:], betaf_sb[:],
                              "bnf", n=3)
    outsb = per.tile([128, C], f32, name="outsb")
    zf = per.tile([128, C], f32, name="zf")
    nc.vector.scalar_tensor_tensor(zf[:], Q_sb[:], s_col[:], P_sb[:],
                                   op0=Alu.mult, op1=Alu.add)
    for mo in range(3):
        fused = per.tile([128, 128], f32, name=f"fused_{mo}")
        nc.scalar.activation(fused[:], zf[:, mo * 128:(mo + 1) * 128],
                             Act.Relu, bias=shf[:, mo:mo + 1],
                             scale=scf[:, mo:mo + 1])
        pst = ps_c.tile([128, 128], f32, name="wtps", tag="cps")
        nc.tensor.transpose(pst[:], fused[:], ident[:])
        nc.vector.tensor_copy(outsb[:, mo * 128:(mo + 1) * 128], pst[:])

    nc.sync.dma_start(io["out"].ap(), outsb[:])
    ctx.close()


def _get_built():
    global _BUILT
    if _BUILT is None:
        _BUILT = _build(DEBUG)
    return _BUILT


def _wt_host(W, rows, cols):
    """Host layout matching wblk(): out[q, (kc*nr+mo)*128+p] =
    W[mo*128+p, kc*128+q]."""
    nr, kg = rows // 128, cols // 128
    blk = W.reshape(nr, 128, kg, 128)          # [mo, p, kc, q]
    return np.ascontiguousarray(
        blk.transpose(3, 2, 0, 1).reshape(128, kg * nr * 128))


def _vec_host(v):
    n = v.shape[0]
    nt = n // 128
    return np.ascontiguousarray(v.reshape(nt, 128).T)


def make_in_maps(inputs):
    pgf = np.ascontiguousarray(
        np.asarray(inputs["point_groups"], dtype=np.float32).reshape(
            1024, NPTS, 3))
    f = lambda n: np.asarray(inputs[n], dtype=np.float32)
    base = {
        "pg_full": pgf,
        "W1": np.ascontiguousarray(f("W1")),
        "W1Th": np.ascontiguousarray(f("W1").T),
        "W2Th": _wt_host(f("W2"), 256, 128),
        "W3Th": _wt_host(f("W3"), 512, 512),
        "W4Th": _wt_host(f("W4"), C, 512),
        "WfTh": _wt_host(f("Wf"), C, 2 * C),
        "vecs": np.ascontiguousarray(np.concatenate(
            [_vec_host(f("b1")), _vec_host(f("gamma1")),
             _vec_host(f("beta1")), _vec_host(f("b2")),
             _vec_host(f("b3")), _vec_host(f("gamma3")),
             _vec_host(f("beta3")), _vec_host(f("b4")),
             _vec_host(f("bf")), _vec_host(f("gammaf")),
             _vec_host(f("betaf")),
             _vec_host(f("beta_aff").reshape(C))], axis=1)),
        "b4row": np.ascontiguousarray(f("b4").reshape(1, C)),
        "alpha_row": np.ascontiguousarray(f("alpha").reshape(1, C)),
    }
    in_maps = []
    for c in range(NCORES):
        m = dict(base)
        pg = pgf[c * G:(c + 1) * G]            # [(t gp), i, c]
        pgr = pg.reshape(32, 4, NPTS, 3)       # [t, gp, i, c]
        m["x0h"] = np.ascontiguousarray(
            pgr.transpose(3, 0, 1, 2).reshape(3, NPOS))
        m["pgAh"] = np.ascontiguousarray(
            pgr.transpose(1, 2, 0, 3).reshape(128, 96))
        m["pgSh"] = np.ascontiguousarray(
            pgr.transpose(1, 0, 2, 3).reshape(4, NPTS * 32 * 3))
        in_maps.append(m)
    return in_maps


def kernel(**inputs):
    from concourse.bass_utils import run_bass_kernel_spmd

    nc = _get_built()
    in_maps = make_in_maps(inputs)
    res = run_bass_kernel_spmd(nc, in_maps, list(range(NCORES)))
    full = np.concatenate([res.results[c]["out"] for c in range(NCORES)],
                          axis=0)
    return full.reshape(4, 256, C)


# revision 66
# speedup vs baseline: 1.0399x; 1.0057x over previous
"""Trainium2 Bass kernel for nn_EnhancedEncoder (gnn_message_passing).

Data-parallel over the 1024 flattened groups: 128 groups per core on 8 cores.
All intermediates stay in SBUF. The KNN gather is reformulated with counts:
U[g] = sum_m (cnt_m - K) feat_m needs only per-position selection counts, so
the H loop has no neighbor-sum matmul on its critical chain; the neighbor sum
hps is kept only for the global T2 moment. fg2 comes from per-tile PE
transposes of position-major feat (conv4 is computed once). Weight/layout
transposes are done host-side in make_in_maps (pure relayout, no arithmetic).
Cross-core reductions: warm-up AllReduce + BN3 stats + one merged final round
(global std of dx + fusion BN stats); the KNN selection phase is declared
between the BN3 collective launch and its consumers so the in-order engine
queues execute it inside the collective's latency window.

Position indexing per core: pos = t*128 + gp*32 + i  (t in [0,32), gp in
[0,4), i in [0,32)); group id g = 4*t + gp.  Channel-major tensors are
[ch_tile(128), pos(4096)]; feat tiles are [4*32 points, 384]; per-group
vectors are [*, g] with g = 4*t + gp.
"""
import sys
from contextlib import ExitStack

for _p in ("/opt/trn_rl_repo",):
    if _p not in sys.path:
        sys.path.insert(0, _p)

import numpy as np

NCORES = 8
G = 128            # groups per core
NPTS = 32          # points per group
NPOS = G * NPTS    # 4096 positions per core
C = 384            # encoder channels
K = 8              # knn group size
EPS = 1e-5
NTOT_POS = 1024 * NPTS          # global positions (BN1/BN3 denominator)
NTOT_DX = 1024 * NPTS * K * C   # global dx element count (std denominator)
NB = 1024                       # global batch of groups (BNf denominator)
BIG_NEG = -1e30
DEBUG = False

_BUILT = None


def _build(debug=False):
    import concourse.bacc as bacc
    import concourse.tile as tile
    from concourse import mybir

    f32 = mybir.dt.float32
    nc = bacc.Bacc("TRN2", target_bir_lowering=False, debug=False,
                   num_devices=NCORES)

    io = {}

    def din(name, shape):
        io[name] = nc.dram_tensor(name, shape, f32, kind="ExternalInput")

    # host-relayouted inputs (pure transpose/reshape of the originals)
    din("x0h", [3, NPOS])          # x0h[c, t*128+gp*32+i] = pg[4t+gp, i, c]
    din("pgAh", [128, 96])         # [32gp+i, (t c)] = pg[4t+gp, i, c]
    din("pgSh", [4, NPTS * 32 * 3])  # [gp, (t m c)] = pg[4t+gp, m, c]
    din("pg_full", [1024, NPTS, 3])
    din("W1", [128, 3])
    din("W1Th", [3, 128])
    din("W2Th", [128, 2 * 128])    # blocks (kc=0, mo) of W2 [256,128]
    din("W3Th", [128, 16 * 128])   # blocks (kc, mo) of W3 [512,512]
    din("W4Th", [128, 12 * 128])   # blocks (kc, mo) of W4 [384,512]
    din("WfTh", [128, 18 * 128])   # blocks (kc, mo) of Wf [384,768]
    din("vecs", [128, 32])         # packed bias/affine columns
    din("b4row", [1, C])
    din("alpha_row", [1, C])
    io["out"] = nc.dram_tensor("out", [G, C], f32, kind="ExternalOutput")
    if debug:
        for nm, sh in [("dbg_f1h", [128, NPOS]), ("dbg_negkey", [128, 1024]),
                       ("dbg_A2", [128, 1024]), ("dbg_Kc", [128, 32]),
                       ("dbg_f3h0", [128, NPOS]), ("dbg_fg2", [128, C]),
                       ("dbg_U", [128, C]), ("dbg_mom", [128, 17]),
                       ("dbg_P", [128, C]), ("dbg_Q", [128, C]),
                       ("dbg_feat0", [128, C]), ("dbg_fg", [128, 256])]:
            io[nm] = nc.dram_tensor(nm, sh, f32, kind="ExternalOutput")

    with tile.TileContext(nc) as tc:
        _emit(nc, tc, tile, mybir, io, debug)
    nc.compile()
    return nc


def _emit(nc, tc, tile, mybir, io, debug):
    f32 = mybir.dt.float32
    f32r = mybir.dt.float32r
    Alu = mybir.AluOpType
    Act = mybir.ActivationFunctionType
    AX = mybir.AxisListType
    RG = [list(range(NCORES))]

    def mm(outap, lhsT, rhs, start, stop, rep=True):
        if rep:
            lhsT = lhsT.bitcast(f32r)
            rhs = rhs.bitcast(f32r)
        nc.tensor.matmul(outap, lhsT, rhs, start=start, stop=stop,
                         skip_group_check=True)

    ctx = ExitStack()
    per = ctx.enter_context(tc.tile_pool(name="per", bufs=1))
    ps_c = ctx.enter_context(tc.tile_pool(name="ps_c", bufs=1, space="PSUM"))
    dram = ctx.enter_context(tc.tile_pool(name="dram", bufs=1, space="DRAM"))

    # ---------------- constants ----------------
    ident = per.tile([128, 128], f32, name="ident")
    nc.gpsimd.memset(ident[:], 1.0)
    nc.gpsimd.affine_select(ident[:], ident[:], pattern=[[1, 128]],
                            compare_op=Alu.is_equal, fill=0.0, base=0,
                            channel_multiplier=-1)
    ones1x128 = per.tile([1, 128], f32, name="ones1x128")
    nc.gpsimd.memset(ones1x128[:], 1.0)
    nc.scalar.activation(ones1x128.bitcast(f32r), ones1x128[:], Act.Identity)
    scr_warm = per.tile([1, 128], f32, name="scr_warm")
    nc.scalar.activation(scr_warm[:], ones1x128[:], Act.Relu)
    nc.scalar.activation(scr_warm[:], ones1x128[:], Act.Square)
    nc.scalar.activation(scr_warm[:], ones1x128[:], Act.Sqrt)
    ones128x1 = per.tile([128, 1], f32, name="ones128x1")
    nc.gpsimd.memset(ones128x1[:], 1.0)
    eps_col = per.tile([128, 1], f32, name="eps_col")
    nc.gpsimd.memset(eps_col[:], EPS)

    # ---------------- dummy collective (comm warm-up) ----------------
    warm_in = dram.tile([128, 1], f32, name="warm_in")
    warm_out = dram.tile([128, 1], f32, name="warm_out")
    nc.sync.dma_start(warm_in[:], ones128x1[:])
    nc.gpsimd.collective_compute("AllReduce", Alu.add, replica_groups=RG,
                                 ins=[warm_in.opt()], outs=[warm_out.opt()])

    # ---------------- load weights + vectors (host pre-laid-out) --------
    # x0 first on the SP queue: conv1 is the critical path
    conv_in_cm = tc.tile_pool(name="conv_in", bufs=1)
    conv_in = conv_in_cm.__enter__()
    x0 = conv_in.tile([3, NPOS], f32, name="x0")
    nc.sync.dma_start(x0[:], io["x0h"].ap())

    vecs = per.tile([128, 32], f32, name="vecs")
    nc.sync.dma_start(vecs[:], io["vecs"].ap())
    b1_sb = vecs[:, 0:1]
    gamma1_sb = vecs[:, 1:2]
    beta1_sb = vecs[:, 2:3]
    b2_sb = vecs[:, 3:5]
    b3_sb = vecs[:, 5:9]
    gamma3_sb = vecs[:, 9:13]
    beta3_sb = vecs[:, 13:17]
    b4_sb = vecs[:, 17:20]
    bf_sb = vecs[:, 20:23]
    gammaf_sb = vecs[:, 23:26]
    betaf_sb = vecs[:, 26:29]
    betaaff_sb = vecs[:, 29:32]
    W1_sb = per.tile([128, 3], f32, name="W1_sb")
    nc.sync.dma_start(W1_sb[:], io["W1"].ap())
    W1T = per.tile([3, 128], f32, name="W1T")
    nc.sync.dma_start(W1T[:], io["W1Th"].ap())
    alpha_row = per.tile([1, C], f32, name="alpha_row")
    nc.sync.dma_start(alpha_row[:], io["alpha_row"].ap())

    # bulk weight loads on the scalar queue so they don't block the SP queue.
    # f32r matmult inputs must come from a rounding instruction, not a DMA:
    # round W2T/W3T/W4T/b4row through the Act engine right after the loads
    # (WfT is only used in plain-f32 matmuls, conv1 runs in plain f32).
    wraw_cm = tc.tile_pool(name="wraw", bufs=1)
    wraw = wraw_cm.__enter__()
    wst = wraw.tile([128, 18 * 128], f32, name="wst")
    W2T = per.tile([128, 2 * 128], f32, name="W2T")
    nc.scalar.dma_start(wst[:, 16 * 128:18 * 128], io["W2Th"].ap())
    nc.scalar.activation(W2T.bitcast(f32r), wst[:, 16 * 128:18 * 128],
                         Act.Identity)
    W3T = per.tile([128, 16 * 128], f32, name="W3T")
    nc.scalar.dma_start(wst[:, :16 * 128], io["W3Th"].ap())
    nc.scalar.activation(W3T.bitcast(f32r), wst[:, :16 * 128], Act.Identity)
    WfT = per.tile([128, 18 * 128], f32, name="WfT")
    nc.scalar.dma_start(WfT[:], io["WfTh"].ap())

    def wblk(wt, nr, kc, mo):
        return wt[:, (kc * nr + mo) * 128:(kc * nr + mo) * 128 + 128]

    def w4rhs(kc):  # pos-major rhs [128, 384] = blocks (kc, mo=0..2)
        return W4T[:, kc * 3 * 128:(kc * 3 + 3) * 128]

    def bn_scale_shift(var_ap, mu_ap, gam_ap, bet_ap, pref, n=1):
        std = per.tile([128, n], f32, name=pref + "_std")
        nc.scalar.activation(std[:], var_ap, Act.Sqrt, bias=eps_col[:])
        rstd = per.tile([128, n], f32, name=pref + "_rstd")
        nc.vector.reciprocal(rstd[:], std[:])
        sc = per.tile([128, n], f32, name=pref + "_sc")
        nc.vector.tensor_tensor(sc[:], rstd[:], gam_ap, op=Alu.mult)
        sh = per.tile([128, n], f32, name=pref + "_sh")
        nc.vector.tensor_tensor(sh[:], mu_ap, sc[:], op=Alu.mult)
        nc.vector.tensor_tensor(sh[:], bet_ap, sh[:], op=Alu.subtract)
        return sc, sh

    # ================ BN1 moments from global input ================
    bn1_cm = tc.tile_pool(name="bn1", bufs=1)
    bn1p = bn1_cm.__enter__()
    pgm = bn1p.tile([128, 768], f32, name="pgm")   # [128, (jj:8, i:32, c:3)]
    nc.gpsimd.dma_start(pgm[:], io["pg_full"].ap().rearrange(
        "(p jj) i c -> p (jj i c)", p=128).opt())
    mcols = bn1p.tile([128, 12], f32, name="mcols")
    pv = pgm.rearrange("p (j c) -> p j c", c=3)
    scr256 = bn1p.tile([128, 256], f32, name="scr256")
    for i in range(3):
        for j in range(3):
            nc.vector.scalar_tensor_tensor(
                scr256[:], pv[:, :, i], 1.0, pv[:, :, j],
                op0=Alu.mult, op1=Alu.mult,
                accum_out=mcols[:, 3 * i + j:3 * i + j + 1])
        nc.vector.tensor_reduce(mcols[:, 9 + i:10 + i], pv[:, :, i],
                                axis=AX.X, op=Alu.add)
    m12 = ps_c.tile([1, 12], f32, name="m12", tag="cps")
    mm(m12[:], ones128x1[:], mcols[:], True, True, rep=False)
    m12s = bn1p.tile([1, 12], f32, name="m12s")
    nc.scalar.activation(m12s[:], m12[:], Act.Identity, scale=1.0 / NTOT_POS)
    M2sb = bn1p.tile([3, 3], f32, name="M2sb")
    nc.gpsimd.dma_start(M2sb[:],
                        m12s[:1, :9].rearrange("1 (i j) -> 1 i j", i=3))
    mu3 = bn1p.tile([3, 1], f32, name="mu3")
    nc.gpsimd.dma_start(mu3[:], m12s[:1, 9:12])

    m1ps = ps_c.tile([128, 1], f32, name="m1ps", tag="cps")
    mm(m1ps[:], W1T[:], mu3[:], True, True, rep=False)   # W1 @ mu_p
    mvec = per.tile([128, 1], f32, name="mvec")
    nc.vector.tensor_copy(mvec[:], m1ps[:])
    wmps = ps_c.tile([128, 3], f32, name="wmps", tag="cps")
    mm(wmps[:], W1T[:], M2sb[:], True, True, rep=False)  # W1 @ M2
    # var of sc1*(W1 x + b1) is translation-invariant: var1 = E2raw - mvec^2
    # and the shift folds to sh1b = beta1 - sc1*mvec (b1 cancels)
    e2raw = per.tile([128, 1], f32, name="e2raw")
    scr3 = per.tile([128, 3], f32, name="scr3")
    nc.vector.scalar_tensor_tensor(scr3[:], wmps[:], 1.0, W1_sb[:],
                                   op0=Alu.mult, op1=Alu.mult,
                                   accum_out=e2raw[:])
    t_a = per.tile([128, 1], f32, name="t_a")
    nc.vector.tensor_tensor(t_a[:], mvec[:], mvec[:], op=Alu.mult)
    var1 = per.tile([128, 1], f32, name="var1")
    nc.vector.tensor_tensor(var1[:], e2raw[:], t_a[:], op=Alu.subtract)
    std1 = per.tile([128, 1], f32, name="std1")
    nc.scalar.activation(std1[:], var1[:], Act.Sqrt, bias=eps_col[:])
    rstd1 = per.tile([128, 1], f32, name="rstd1")
    nc.vector.reciprocal(rstd1[:], std1[:])
    sc1 = per.tile([128, 1], f32, name="sc1")
    nc.vector.tensor_tensor(sc1[:], rstd1[:], gamma1_sb[:], op=Alu.mult)
    sh1b = per.tile([128, 1], f32, name="sh1b")
    nc.vector.tensor_tensor(sh1b[:], mvec[:], sc1[:], op=Alu.mult)
    nc.vector.tensor_tensor(sh1b[:], beta1_sb[:], sh1b[:], op=Alu.subtract)
    bn1_cm.__exit__(None, None, None)

    # early DMAs for the selection phase (consumed later)
    selin_cm = tc.tile_pool(name="selin", bufs=1)
    selin = selin_cm.__enter__()
    pgA = selin.tile([128, 96], f32, name="pgA")
    nc.gpsimd.dma_start(pgA[:], io["pgAh"].ap())
    pgB = selin.tile([128, 3072], f32, name="pgB")
    for gp in range(4):
        nc.gpsimd.dma_start(
            pgB[32 * gp:32 * gp + 32],
            io["pgSh"].ap()[gp:gp + 1].broadcast_to([32, 3072]))


    # ---------------- data-independent H-phase constants ----------------
    # onesblk[32gp+n, gp'] = 1 iff gp' == gp; stationary for per-t U writes
    onesblk = per.tile([128, 4], f32, name="onesblk")
    nc.gpsimd.memset(onesblk[:], 1.0)
    nc.gpsimd.affine_select(onesblk[:], onesblk[:], pattern=[[-32, 4]],
                            compare_op=Alu.is_ge, fill=0.0, base=0,
                            channel_multiplier=1)
    nc.gpsimd.affine_select(onesblk[:], onesblk[:], pattern=[[32, 4]],
                            compare_op=Alu.is_ge, fill=0.0, base=31,
                            channel_multiplier=-1)
    alphar_ps = ps_c.tile([128, C], f32, name="alphar_ps", tag="cps")
    alpha_row = per.tile([1, C], f32, name="alpha_row")
    nc.sync.dma_start(alpha_row[:], io["alpha_row"].ap())

    # ================ conv1 / conv2 ================
    ps_mm_cm = tc.tile_pool(name="ps_mm", bufs=6, space="PSUM")
    ps_mm = ps_mm_cm.__enter__()
    act3_cm = tc.tile_pool(name="act3", bufs=1)
    act3 = act3_cm.__enter__()
    act1_cm = tc.tile_pool(name="act1", bufs=1)
    act1 = act1_cm.__enter__()

    f1h = act1.tile([128, NPOS], f32, name="f1h")
    for nt in range(8):
        ps = ps_mm.tile([128, 512], f32, name="mmps", tag="mmps")
        mm(ps[:], W1T[:], x0[:, nt * 512:(nt + 1) * 512], True, True,
           rep=False)
        nc.scalar.activation(f1h[:, nt * 512:(nt + 1) * 512].bitcast(f32r),
                             ps[:], Act.Relu, bias=sh1b[:], scale=sc1[:])
    if debug:
        nc.sync.dma_start(io["dbg_f1h"].ap(), f1h[:])

    fg = per.tile([128, 256], f32, name="fg")  # [128, (mo:2, g:128)]
    f2 = [act3.tile([128, NPOS], f32, name=f"f2_{mo}") for mo in range(2)]
    for mo in range(2):
        for nt in range(8):
            ps = ps_mm.tile([128, 512], f32, name="mmps", tag="mmps")
            mm(ps[:], wblk(W2T, 2, 0, mo), f1h[:, nt * 512:(nt + 1) * 512],
               True, True)
            nc.scalar.activation(
                f2[mo][:, nt * 512:(nt + 1) * 512].bitcast(f32r), ps[:],
                Act.Identity, bias=b2_sb[:, mo:mo + 1])
            # per-group max per chunk (16 groups) so fg is ready for conv3
            nc.vector.tensor_reduce(
                fg[:, mo * 128 + nt * 16:mo * 128 + (nt + 1) * 16]
                .bitcast(f32r),
                f2[mo][:, nt * 512:(nt + 1) * 512]
                .rearrange("p (g i) -> p g i", i=32),
                axis=AX.X, op=Alu.max)
    act1_cm.__exit__(None, None, None)
    if debug:
        nc.sync.dma_start(io["dbg_fg"].ap(), fg[:])

    # ================ conv3 (stats in, bias copy out) ================
    f3 = [per.tile([128, NPOS], f32, name=f"f3_{mo}") for mo in range(4)]
    stats3 = per.tile([128, 4 * 8 * 6], f32, name="stats3")
    mv3 = per.tile([128, 8], f32, name="mv3")
    for mo in range(4):
        for ntc in range(4):
            pss = [ps_mm.tile([128, 512], f32, name="mmps", tag="mmps")
                   for _ in range(2)]
            # f2 blocks first so the fg reduce is off the critical path
            for kc in (2, 3, 0, 1):
                for j, nt in enumerate((2 * ntc, 2 * ntc + 1)):
                    if kc < 2:
                        rhs = fg[:, kc * 128 + nt * 16:
                                 kc * 128 + (nt + 1) * 16] \
                            .unsqueeze(2).broadcast_to([128, 16, 32])
                    else:
                        rhs = f2[kc - 2][:, nt * 512:(nt + 1) * 512]
                    mm(pss[j][:], wblk(W3T, 4, kc, mo), rhs, kc == 2,
                       kc == 1)
            for j, nt in enumerate((2 * ntc, 2 * ntc + 1)):
                dst = f3[mo][:, nt * 512:(nt + 1) * 512].bitcast(f32r)
                nc.scalar.activation(dst, pss[j][:], Act.Identity,
                                     bias=b3_sb[:, mo:mo + 1])
                nc.vector.bn_stats(
                    stats3[:, (mo * 8 + nt) * 6:(mo * 8 + nt) * 6 + 6],
                    dst)
        nc.vector.bn_aggr(mv3[:, mo * 2:mo * 2 + 2],
                          stats3[:, mo * 48:(mo + 1) * 48])
    act3_cm.__exit__(None, None, None)
    ps_mm_cm.__exit__(None, None, None)

    # local (sum, sumsq) per channel -> AllReduce (launch ASAP)
    # psum stats lack +b3, but b3 cancels in the variance; ship raw
    # sums/sumsq and add b3 to the global mean after the AllReduce
    bnloc = per.tile([128, 8], f32, name="bnloc")
    mv3v = mv3.rearrange("p (m two) -> p two m", two=2)
    bnlv = bnloc.rearrange("p (m two) -> p two m", two=2)
    nc.vector.tensor_scalar(bnlv[:, 0, :], mv3v[:, 0, :], float(NPOS), None,
                            op0=Alu.mult)
    scrb3 = per.tile([128, 4], f32, name="scrb3")
    nc.vector.scalar_tensor_tensor(scrb3[:], mv3v[:, 0, :], 1.0,
                                   mv3v[:, 0, :], op0=Alu.mult, op1=Alu.mult)
    nc.vector.tensor_tensor(scrb3[:], scrb3[:], mv3v[:, 1, :], op=Alu.add)
    nc.vector.tensor_scalar(bnlv[:, 1, :], scrb3[:], float(NPOS), None,
                            op0=Alu.mult)
    cc3_in = dram.tile([128, 8], f32, name="cc3_in")
    cc3_out = dram.tile([128, 8], f32, name="cc3_out")
    nc.sync.dma_start(cc3_in[:], bnloc[:])
    nc.gpsimd.collective_compute("AllReduce", Alu.add, replica_groups=RG,
                                 ins=[cc3_in.opt()], outs=[cc3_out.opt()])

    W4T = per.tile([128, 12 * 128], f32, name="W4T")
    nc.scalar.dma_start(wst[:, :12 * 128], io["W4Th"].ap())
    nc.scalar.activation(W4T.bitcast(f32r), wst[:, :12 * 128], Act.Identity)
    b4row0 = per.tile([1, C], f32, name="b4row0")
    nc.sync.dma_start(b4row0[:], io["b4row"].ap())
    b4row = per.tile([1, C], f32, name="b4row")
    nc.scalar.activation(b4row.bitcast(f32r), b4row0[:], Act.Identity)
    # wbias = Wf[:, C:] @ beta_aff + bf   (channel-major [128, 3])
    wbias_ps = ps_c.tile([128, 3], f32, name="wbias_ps", tag="cps")
    for mo in range(3):
        for kc in range(3):
            mm(wbias_ps[:, mo:mo + 1], wblk(WfT, 3, 3 + kc, mo),
               betaaff_sb[:, kc:kc + 1], kc == 0, kc == 2, rep=False)
    wbias = per.tile([128, 3], f32, name="wbias")
    nc.vector.tensor_tensor(wbias[:], wbias_ps[:], bf_sb[:], op=Alu.add)

    mm(alphar_ps[:], ones1x128[:], alpha_row[:], True, True, rep=False)
    alphar = per.tile([128, C], f32, name="alphar")
    nc.scalar.activation(alphar[:], alphar_ps[:], Act.Identity)


    # ====== selection, declared here so it runs inside the BN3 window ======
    hconst_cm = tc.tile_pool(name="hconst", bufs=1)
    hc = hconst_cm.__enter__()
    # W_B zero background (block-diag A2T copied in below)
    W_B = hc.tile([128, NPOS], f32, name="W_B")
    nc.gpsimd.memset(W_B[:], 0.0)
    nc.gpsimd.tensor_copy(W_B.bitcast(f32r), W_B[:])
    # onesU[32*gp+n, t*128 + m] = 1 iff m == 4t+gp
    onesU = hc.tile([128, 32 * 128], f32, name="onesU")
    nc.gpsimd.memset(onesU[:], 0.0)
    nc.gpsimd.tensor_copy(onesU.bitcast(f32r), onesU[:])
    for t in range(32):
        nc.gpsimd.tensor_copy(
            onesU[:, t * 128 + 4 * t:t * 128 + 4 * t + 4].bitcast(f32r),
            onesblk[:])
    sel_b = tc.tile_pool(name="sel_b", bufs=1)
    sb = sel_b.__enter__()
    # negkey[32gp+n, t*32+m] = sum_c (pgA[.,t,c] - 0.5*pgB_c)*pgB_c
    scr1 = sb.tile([128, 1024], f32, name="scr1")
    negkey = sb.tile([128, 1024], f32, name="negkey")
    for cdim in range(3):
        pgB_c = pgB.rearrange("p (t m c) -> p t m c", t=32, m=32)[:, :, :, cdim]
        pgA_c = pgA.rearrange("p (t c) -> p t c", c=3)[:, :, cdim] \
            .unsqueeze(2).broadcast_to([128, 32, 32])
        dst = scr1[:] if cdim > 0 else negkey[:]
        dstv = dst.rearrange("p (t m) -> p t m", t=32)
        nc.vector.scalar_tensor_tensor(dstv, pgB_c, -0.5, pgA_c,
                                       op0=Alu.mult, op1=Alu.add)
        nc.vector.tensor_tensor(dstv, dstv, pgB_c, op=Alu.mult)
        if cdim > 0:
            nc.vector.tensor_tensor(negkey[:], negkey[:], scr1[:],
                                    op=Alu.add)

    top8 = sb.tile([128, 8], f32, name="top8")
    repl = sb.tile([128, 1024], f32, name="repl", tag="repl")
    for t in range(32):
        nc.vector.max(top8[:], negkey[:, t * 32:(t + 1) * 32])
        nc.vector.match_replace(repl[:, t * 32:(t + 1) * 32], top8[:],
                                negkey[:, t * 32:(t + 1) * 32], BIG_NEG)
    A2 = sb.tile([128, 1024], f32, name="A2")
    nc.vector.tensor_scalar(A2[:], repl[:], BIG_NEG, None, op0=Alu.is_equal)
    if debug:
        nc.sync.dma_start(io["dbg_negkey"].ap(), negkey[:])
        nc.sync.dma_start(io["dbg_A2"].ap(), A2[:])

    A2T = sb.tile([128, 1024], f32, name="A2T", tag="repl")
    nc.vector.transpose(A2T[:], A2[:])
    # Kc[32gp+m, t] = K + sum_n A[n, m];  Kw = Kc - 2K (U weights)
    Kc = per.tile([128, 32], f32, name="Kc")
    nc.vector.tensor_reduce(Kc[:],
                            A2T.rearrange("p (t n) -> p t n", t=32),
                            axis=AX.X, op=Alu.add)
    nc.vector.tensor_scalar(Kc[:], Kc[:], float(K), None, op0=Alu.add)
    Kw = per.tile([128, 32], f32, name="Kw")
    nc.vector.tensor_scalar(Kw[:], Kc[:], -2.0 * K, None, op0=Alu.add)
    if debug:
        nc.sync.dma_start(io["dbg_Kc"].ap(), Kc[:])

    # W_B[32gp+m, t*128+32gp+n] = A2T[32gp+m, t*32+n]  (block-diag lhsT)
    for gp in range(4):
        nc.vector.tensor_copy(
            W_B[32 * gp:32 * gp + 32].rearrange(
                "p (t q) -> p t q", t=32)[:, :, 32 * gp:32 * gp + 32]
            .bitcast(f32r),
            A2T[32 * gp:32 * gp + 32].rearrange("p (t n) -> p t n", t=32))
    sel_b.__exit__(None, None, None)

    # ====== BN3 post-collective scale/shift + chunked ReLU3 ======
    g3 = per.tile([128, 8], f32, name="g3")
    nc.sync.dma_start(g3[:], cc3_out[:])
    gmu3 = per.tile([128, 4], f32, name="gmu3")
    nc.scalar.activation(gmu3[:], g3.rearrange("p (m two) -> p two m",
                                               two=2)[:, 0, :],
                         Act.Identity, scale=1.0 / NTOT_POS)
    ge23 = per.tile([128, 4], f32, name="ge23")
    nc.scalar.activation(ge23[:], g3.rearrange("p (m two) -> p two m",
                                               two=2)[:, 1, :],
                         Act.Identity, scale=1.0 / NTOT_POS)
    gvar3 = per.tile([128, 4], f32, name="gvar3")
    nc.vector.tensor_tensor(gvar3[:], gmu3[:], gmu3[:], op=Alu.mult)
    nc.vector.tensor_tensor(gvar3[:], ge23[:], gvar3[:], op=Alu.subtract)
    sc3, sh3 = bn_scale_shift(gvar3[:], gmu3[:], gamma3_sb[:], beta3_sb[:],
                              "bn3", n=4)

    # ReLU3 chunk nt covers H iterations t in [4nt, 4nt+4); interleave the
    # chunks into the H loop so the in-order Act queue doesn't drain all of
    # ReLU3 before feat t=0
    def relu3_chunk(nt, eng="pool"):
        for mo in range(4):
            sl = f3[mo][:, nt * 512:(nt + 1) * 512]
            if eng == "act":
                nc.scalar.activation(sl.bitcast(f32r), sl, Act.Relu,
                                     bias=sh3[:, mo:mo + 1],
                                     scale=sc3[:, mo:mo + 1])
            else:
                nc.gpsimd.tensor_scalar(sl.bitcast(f32r), sl,
                                        sc3[:, mo:mo + 1], sh3[:, mo:mo + 1],
                                        op0=Alu.mult, op1=Alu.add)
                nc.gpsimd.tensor_scalar(sl.bitcast(f32r), sl.bitcast(f32r),
                                        0.0, None, op0=Alu.max)

    # ================ H phase: conv4 pos-major, U, moments, fg2 =========
    fg2 = per.tile([128, C], f32, name="fg2")     # [128ch, (mo:3, g:128)]
    fg2v = fg2.rearrange("p (mo g) -> p mo g", mo=3)
    sqcol = per.tile([128, 32], f32, name="sqcol")  # ||feat_pos||^2 per t
    acc2 = per.tile([128, 32], f32, name="acc2")    # feat . h per t
    scrSq = per.tile([128, C], f32, name="scrSq")
    scrH = per.tile([128, C], f32, name="scrH")
    U_sb = per.tile([128, C], f32, name="U_sb")
    t1col = per.tile([128, 1], f32, name="t1col")

    with tc.tile_pool(name="psU", bufs=1, space="PSUM") as psU:
        Ups = psU.tile([128, C], f32, name="Ups", tag="hold")
        with tc.tile_pool(name="featpool", bufs=4) as featpool, \
             tc.tile_pool(name="psF", bufs=3, space="PSUM") as psF, \
             tc.tile_pool(name="psT", bufs=2, space="PSUM") as psT:
            for mo in range(2):
                sl = f3[mo][:, 0:512]
                nc.scalar.activation(sl.bitcast(f32r), sl, Act.Relu,
                                     bias=sh3[:, mo:mo + 1],
                                     scale=sc3[:, mo:mo + 1])
            for mo in range(2, 4):
                sl = f3[mo][:, 0:512]
                nc.vector.tensor_scalar(sl.bitcast(f32r), sl,
                                        sc3[:, mo:mo + 1], sh3[:, mo:mo + 1],
                                        op0=Alu.mult, op1=Alu.add)
                nc.vector.tensor_scalar(sl.bitcast(f32r), sl.bitcast(f32r),
                                        0.0, None, op0=Alu.max)
            relu3_chunk(1)
            for t in range(32):
                fps = psF.tile([128, C], f32, name="fps", tag="fps")
                for kc in range(4):
                    mm(fps[:], f3[kc][:, t * 128:(t + 1) * 128], w4rhs(kc),
                       kc == 0, False)
                mm(fps[:], ones1x128[:], b4row[:], False, True)  # + b4
                feat = featpool.tile([128, C], f32, name="feat", tag="feat")
                nc.scalar.activation(feat.bitcast(f32r), fps[:], Act.Identity)
                if debug and t == 0:
                    nc.sync.dma_start(io["dbg_feat0"].ap(), feat[:])
                # ||feat||^2 per position (from SBUF so fps frees earlier)
                nc.scalar.activation(scrSq[:], feat[:], Act.Square,
                                     accum_out=sqcol[:, t:t + 1])
                # fg2 via PE transpose (f32r) + combined per-group max
                pst = psT.tile([128, C], f32, name="tps", tag="tps")
                for mo in range(3):
                    nc.tensor.matmul(
                        pst[:, mo * 128:(mo + 1) * 128],
                        feat[:, mo * 128:(mo + 1) * 128],
                        ident[:], is_transpose=True,
                        skip_group_check=True)
                nc.vector.tensor_reduce(
                    fg2v[:, :, 4 * t:4 * t + 4],
                    pst.rearrange("p (mo gp i) -> p mo gp i", mo=3, i=32),
                    axis=AX.X, op=Alu.max)
                # neighbor sum (for the T2 moment only)
                hps = psF.tile([128, C], f32, name="hps", tag="hps",
                               bufs=1)
                mm(hps[:], W_B[:, t * 128:(t + 1) * 128], feat[:],
                   True, True)
                nc.vector.scalar_tensor_tensor(
                    scrH[:], feat[:], 1.0, hps[:],
                    op0=Alu.mult, op1=Alu.mult, accum_out=acc2[:, t:t + 1])
                # U path: wfeat = (Kc - 2K) * feat
                wfeat = featpool.tile([128, C], f32, name="wfeat", tag="wf")
                nc.vector.tensor_scalar(wfeat.bitcast(f32r), feat[:],
                                        Kw[:, t:t + 1], None, op0=Alu.mult)
                mm(Ups[:], onesU[:, t * 128:(t + 1) * 128], wfeat[:],
                   t == 0, t == 31)
                if t % 4 == 0 and t // 4 + 2 < 8:
                    nt = t // 4 + 2
                    relu3_chunk(nt, eng="act" if nt % 2 == 1 else "pool")
        nc.scalar.activation(U_sb[:], Ups[:], Act.Identity,
                             accum_out=t1col[:])
    hconst_cm.__exit__(None, None, None)
    selin_cm.__exit__(None, None, None)
    wraw_cm.__exit__(None, None, None)
    conv_in_cm.__exit__(None, None, None)
    if debug:
        nc.sync.dma_start(io["dbg_fg2"].ap(), fg2[:])

    # t2col = sum_t (Kc*sq) - 2*sum_t acc2
    a1r = per.tile([128, 1], f32, name="a1r")
    scr32 = per.tile([128, 32], f32, name="scr32")
    nc.vector.scalar_tensor_tensor(scr32[:], sqcol[:], 1.0, Kc[:],
                                   op0=Alu.mult, op1=Alu.mult,
                                   accum_out=a1r[:])
    a2r = per.tile([128, 1], f32, name="a2r")
    nc.vector.tensor_reduce(a2r[:], acc2[:], axis=AX.X, op=Alu.add)
    t2col = per.tile([128, 1], f32, name="t2col")
    nc.vector.scalar_tensor_tensor(t2col[:], a2r[:], -2.0, a1r[:],
                                   op0=Alu.mult, op1=Alu.add)

    # V = alpha * U / (n*K)  (group-major), then transpose to channel-major
    V_sb = per.tile([128, C], f32, name="V_sb")
    nc.vector.scalar_tensor_tensor(V_sb[:], U_sb[:], 1.0 / (NPTS * K),
                                   alphar[:], op0=Alu.mult, op1=Alu.mult)
    Vc = per.tile([128, C], f32, name="Vc")
    for mo in range(3):
        pstv = ps_c.tile([128, 128], f32, name="wtps", tag="cps")
        nc.tensor.transpose(pstv[:], V_sb[:, mo * 128:(mo + 1) * 128],
                            ident[:])
        nc.vector.tensor_copy(Vc[:, mo * 128:(mo + 1) * 128], pstv[:])
    if debug:
        nc.sync.dma_start(io["dbg_U"].ap(), U_sb[:])

    # ================ P/Q matmuls + moments ================
    P_sb = per.tile([128, C], f32, name="P_sb")
    Q_sb = per.tile([128, C], f32, name="Q_sb")
    mom = per.tile([128, 17], f32, name="mom")
    scrP = per.tile([128, 128], f32, name="scrP")
    with tc.tile_pool(name="psQ", bufs=1, space="PSUM") as psQ:
        Pps = psQ.tile([128, C], f32, name="Pps", tag="holdP")
        Qps = psQ.tile([128, C], f32, name="Qps", tag="holdQ")
        for mo in range(3):
            for kc in range(3):
                mm(Pps[:, mo * 128:(mo + 1) * 128], wblk(WfT, 3, kc, mo),
                   fg2[:, kc * 128:(kc + 1) * 128], kc == 0, kc == 2,
                   rep=False)
        for mo in range(3):
            for kc in range(3):
                mm(Qps[:, mo * 128:(mo + 1) * 128], wblk(WfT, 3, 3 + kc, mo),
                   Vc[:, kc * 128:(kc + 1) * 128], kc == 0, kc == 2,
                   rep=False)
        scrQ = per.tile([128, 128], f32, name="scrQ")
        scrPQ = per.tile([128, 128], f32, name="scrPQ")
        for mo in range(3):
            nc.scalar.activation(P_sb[:, mo * 128:(mo + 1) * 128],
                                 Pps[:, mo * 128:(mo + 1) * 128],
                                 Act.Identity, bias=wbias[:, mo:mo + 1],
                                 accum_out=mom[:, mo:mo + 1])
            nc.scalar.activation(Q_sb[:, mo * 128:(mo + 1) * 128],
                                 Qps[:, mo * 128:(mo + 1) * 128], Act.Identity,
                                 accum_out=mom[:, 3 + mo:4 + mo])
            nc.scalar.activation(scrP[:], P_sb[:, mo * 128:(mo + 1) * 128],
                                 Act.Square, accum_out=mom[:, 6 + mo:7 + mo])
            nc.vector.scalar_tensor_tensor(
                scrQ[:], Q_sb[:, mo * 128:(mo + 1) * 128], 1.0,
                Q_sb[:, mo * 128:(mo + 1) * 128], op0=Alu.mult, op1=Alu.mult,
                accum_out=mom[:, 9 + mo:10 + mo])
            nc.vector.scalar_tensor_tensor(
                scrPQ[:], P_sb[:, mo * 128:(mo + 1) * 128], 1.0,
                Q_sb[:, mo * 128:(mo + 1) * 128], op0=Alu.mult, op1=Alu.mult,
                accum_out=mom[:, 12 + mo:13 + mo])
    # partition-sum t1/t2 and broadcast before the AllReduce so gmom holds
    # the global scalars directly on every partition
    t12l = per.tile([128, 2], f32, name="t12l")
    nc.vector.tensor_copy(t12l[:, 0:1], t1col[:])
    nc.vector.tensor_copy(t12l[:, 1:2], t2col[:])
    t12_ps = ps_c.tile([1, 2], f32, name="t12_ps", tag="cps")
    mm(t12_ps[:], ones128x1[:], t12l[:], True, True, rep=False)
    t12 = per.tile([1, 2], f32, name="t12")
    nc.vector.tensor_copy(t12[:], t12_ps[:])
    t12b_ps = ps_c.tile([128, 2], f32, name="t12b_ps", tag="cps")
    mm(t12b_ps[:], ones1x128[:], t12[:], True, True, rep=False)
    nc.vector.tensor_copy(mom[:, 15:17], t12b_ps[:])
    if debug:
        nc.sync.dma_start(io["dbg_P"].ap(), P_sb[:])
        nc.sync.dma_start(io["dbg_Q"].ap(), Q_sb[:])
        nc.sync.dma_start(io["dbg_mom"].ap(), mom[:])

    ccf_in = dram.tile([128, 17], f32, name="ccf_in")
    ccf_out = dram.tile([128, 17], f32, name="ccf_out")
    nc.sync.dma_start(ccf_in[:], mom[:])
    nc.gpsimd.collective_compute("AllReduce", Alu.add, replica_groups=RG,
                                 ins=[ccf_in.opt()], outs=[ccf_out.opt()])
    gmom = per.tile([128, 17], f32, name="gmom")
    nc.sync.dma_start(gmom[:], ccf_out[:])

    T1 = gmom[:, 15:16]
    T2 = gmom[:, 16:17]

    # s = 1 / (std + EPS); var = (T2 - T1^2/N) / (N-1)
    tA = per.tile([128, 1], f32, name="tA")
    nc.vector.tensor_tensor(tA[:], T1, T1, op=Alu.mult)
    tB = per.tile([128, 1], f32, name="tB")
    nc.vector.scalar_tensor_tensor(tB[:], tA[:], -1.0 / NTOT_DX, T2,
                                   op0=Alu.mult, op1=Alu.add)
    stdx = per.tile([128, 1], f32, name="stdx")
    nc.scalar.activation(stdx[:], tB[:], Act.Sqrt,
                         scale=1.0 / (NTOT_DX - 1))
    nc.vector.tensor_scalar(stdx[:], stdx[:], EPS, None, op0=Alu.add)
    s_col = per.tile([128, 1], f32, name="s_col")
    nc.vector.reciprocal(s_col[:], stdx[:])
    s2_col = per.tile([128, 1], f32, name="s2_col")
    nc.vector.tensor_tensor(s2_col[:], s_col[:], s_col[:], op=Alu.mult)
    ts2 = per.tile([128, 1], f32, name="ts2")
    nc.vector.tensor_scalar(ts2[:], s_col[:], 2.0, None, op0=Alu.mult)

    # ================ BNf + output (vectorized over mo) ================
    # muf = (sumP + s*sumQ) / NB
    muf = per.tile([128, 3], f32, name="muf")
    nc.vector.scalar_tensor_tensor(muf[:], gmom[:, 3:6], s_col[:],
                                   gmom[:, 0:3], op0=Alu.mult, op1=Alu.add)
    nc.scalar.activation(muf[:], muf[:], Act.Identity, scale=1.0 / NB)
    # e2f = (sumP2 + 2s*sumPQ + s^2*sumQ2) / NB
    e2f = per.tile([128, 3], f32, name="e2f")
    nc.vector.scalar_tensor_tensor(e2f[:], gmom[:, 12:15], ts2[:],
                                   gmom[:, 6:9], op0=Alu.mult, op1=Alu.add)
    nc.vector.scalar_tensor_tensor(e2f[:], gmom[:, 9:12], s2_col[:],
                                   e2f[:], op0=Alu.mult, op1=Alu.add)
    nc.scalar.activation(e2f[:], e2f[:], Act.Identity, scale=1.0 / NB)
    varf = per.tile([128, 3], f32, name="varf")
    nc.vector.tensor_tensor(varf[:], muf[:], muf[:], op=Alu.mult)
    nc.vector.tensor_tensor(varf[:], e2f[:], varf[:], op=Alu.subtract)
    scf, shf = bn_scale_shift(varf[:], muf[:], gammaf_sb[:], betaf_sb[:],
                              "bnf", n=3)
    outsb = per.tile([128, C], f32, name="outsb")
    zf = per.tile([128, C], f32, name="zf")
    nc.vector.scalar_tensor_tensor(zf[:], Q_sb[:], s_col[:], P_sb[:],
                                   op0=Alu.mult, op1=Alu.add)
    for mo in range(3):
        fused = per.tile([128, 128], f32, name=f"fused_{mo}")
        nc.scalar.activation(fused[:], zf[:, mo * 128:(mo + 1) * 128],
                             Act.Relu, bias=shf[:, mo:mo + 1],
                             scale=scf[:, mo:mo + 1])
        pst = ps_c.tile([128, 128], f32, name="wtps", tag="cps")
        nc.tensor.transpose(pst[:], fused[:], ident[:])
        nc.vector.tensor_copy(outsb[:, mo * 128:(mo + 1) * 128], pst[:])

    nc.sync.dma_start(io["out"].ap(), outsb[:])
    ctx.close()


def _get_built():
    global _BUILT
    if _BUILT is None:
        _BUILT = _build(DEBUG)
    return _BUILT


def _wt_host(W, rows, cols):
    """Host layout matching wblk(): out[q, (kc*nr+mo)*128+p] =
    W[mo*128+p, kc*128+q]."""
    nr, kg = rows // 128, cols // 128
    blk = W.reshape(nr, 128, kg, 128)          # [mo, p, kc, q]
    return np.ascontiguousarray(
        blk.transpose(3, 2, 0, 1).reshape(128, kg * nr * 128))


def _vec_host(v):
    n = v.shape[0]
    nt = n // 128
    return np.ascontiguousarray(v.reshape(nt, 128).T)


def make_in_maps(inputs):
    pgf = np.ascontiguousarray(
        np.asarray(inputs["point_groups"], dtype=np.float32).reshape(
            1024, NPTS, 3))
    f = lambda n: np.asarray(inputs[n], dtype=np.float32)
    base = {
        "pg_full": pgf,
        "W1": np.ascontiguousarray(f("W1")),
        "W1Th": np.ascontiguousarray(f("W1").T),
        "W2Th": _wt_host(f("W2"), 256, 128),
        "W3Th": _wt_host(f("W3"), 512, 512),
        "W4Th": _wt_host(f("W4"), C, 512),
        "WfTh": _wt_host(f("Wf"), C, 2 * C),
        "vecs": np.ascontiguousarray(np.concatenate(
            [_vec_host(f("b1")), _vec_host(f("gamma1")),
             _vec_host(f("beta1")), _vec_host(f("b2")),
             _vec_host(f("b3")), _vec_host(f("gamma3")),
             _vec_host(f("beta3")), _vec_host(f("b4")),
             _vec_host(f("bf")), _vec_host(f("gammaf")),
             _vec_host(f("betaf")),
             _vec_host(f("beta_aff").reshape(C))], axis=1)),
        "b4row": np.ascontiguousarray(f("b4").reshape(1, C)),
        "alpha_row": np.ascontiguousarray(f("alpha").reshape(1, C)),
    }
    in_maps = []
    for c in range(NCORES):
        m = dict(base)
        pg = pgf[c * G:(c + 1) * G]            # [(t gp), i, c]
        pgr = pg.reshape(32, 4, NPTS, 3)       # [t, gp, i, c]
        m["x0h"] = np.ascontiguousarray(
            pgr.transpose(3, 0, 1, 2).reshape(3, NPOS))
        m["pgAh"] = np.ascontiguousarray(
            pgr.transpose(1, 2, 0, 3).reshape(128, 96))
        m["pgSh"] = np.ascontiguousarray(
            pgr.transpose(1, 0, 2, 3).reshape(4, NPTS * 32 * 3))
        in_maps.append(m)
    return in_maps


def kernel(**inputs):
    from concourse.bass_utils import run_bass_kernel_spmd

    nc = _get_built()
    in_maps = make_in_maps(inputs)
    res = run_bass_kernel_spmd(nc, in_maps, list(range(NCORES)))
    full = np.concatenate([res.results[c]["out"] for c in range(NCORES)],
                          axis=0)
    return full.reshape(4, 256, C)


# revision 71
# speedup vs baseline: 1.0576x; 1.0170x over previous
"""Trainium2 Bass kernel for nn_EnhancedEncoder (gnn_message_passing).

Data-parallel over the 1024 flattened groups: 128 groups per core on 8 cores.
All intermediates stay in SBUF. The KNN gather is reformulated with counts:
U[g] = sum_m (cnt_m - K) feat_m needs only per-position selection counts, so
the H loop has no neighbor-sum matmul on its critical chain; the neighbor sum
hps is kept only for the global T2 moment. fg2 comes from per-tile PE
transposes of position-major feat (conv4 is computed once). Weight/layout
transposes are done host-side in make_in_maps (pure relayout, no arithmetic).
Cross-core reductions: warm-up AllReduce + BN3 stats + one merged final round
(global std of dx + fusion BN stats); the KNN selection phase is declared
between the BN3 collective launch and its consumers so the in-order engine
queues execute it inside the collective's latency window.

Position indexing per core: pos = t*128 + gp*32 + i  (t in [0,32), gp in
[0,4), i in [0,32)); group id g = 4*t + gp.  Channel-major tensors are
[ch_tile(128), pos(4096)]; feat tiles are [4*32 points, 384]; per-group
vectors are [*, g] with g = 4*t + gp.
"""
import sys
from contextlib import ExitStack

for _p in ("/opt/trn_rl_repo",):
    if _p not in sys.path:
        sys.path.insert(0, _p)

import numpy as np

NCORES = 8
G = 128            # groups per core
NPTS = 32          # points per group
NPOS = G * NPTS    # 4096 positions per core
C = 384            # encoder channels
K = 8              # knn group size
EPS = 1e-5
NTOT_POS = 1024 * NPTS          # global positions (BN1/BN3 denominator)
NTOT_DX = 1024 * NPTS * K * C   # global dx element count (std denominator)
NB = 1024                       # global batch of groups (BNf denominator)
BIG_NEG = -1e30
DEBUG = False

_BUILT = None


def _build(debug=False):
    import concourse.bacc as bacc
    import concourse.tile as tile
    from concourse import mybir

    f32 = mybir.dt.float32
    nc = bacc.Bacc("TRN2", target_bir_lowering=False, debug=False,
                   num_devices=NCORES)

    io = {}

    def din(name, shape):
        io[name] = nc.dram_tensor(name, shape, f32, kind="ExternalInput")

    # host-relayouted inputs (pure transpose/reshape of the originals)
    din("x0h", [3, NPOS])          # x0h[c, t*128+gp*32+i] = pg[4t+gp, i, c]
    din("pgAh", [128, 96])         # [32gp+i, (t c)] = pg[4t+gp, i, c]
    din("pgSh", [4, NPTS * 32 * 3])  # [gp, (t m c)] = pg[4t+gp, m, c]
    din("pg_full", [1024, NPTS, 3])
    din("W1", [128, 3])
    din("W1Th", [3, 128])
    din("W2Th", [128, 2 * 128])    # blocks (kc=0, mo) of W2 [256,128]
    din("W3Th", [128, 16 * 128])   # blocks (kc, mo) of W3 [512,512]
    din("W4Th", [128, 12 * 128])   # blocks (kc, mo) of W4 [384,512]
    din("WfTh", [128, 18 * 128])   # blocks (kc, mo) of Wf [384,768]
    din("vecs", [128, 32])         # packed bias/affine columns
    din("b4row", [1, C])
    din("alpha_row", [1, C])
    io["out"] = nc.dram_tensor("out", [G, C], f32, kind="ExternalOutput")
    if debug:
        for nm, sh in [("dbg_f1h", [128, NPOS]), ("dbg_negkey", [128, 1024]),
                       ("dbg_A2", [128, 1024]), ("dbg_Kc", [128, 32]),
                       ("dbg_f3h0", [128, NPOS]), ("dbg_fg2", [128, C]),
                       ("dbg_U", [128, C]), ("dbg_mom", [128, 17]),
                       ("dbg_P", [128, C]), ("dbg_Q", [128, C]),
                       ("dbg_feat0", [128, C]), ("dbg_fg", [128, 256])]:
            io[nm] = nc.dram_tensor(nm, sh, f32, kind="ExternalOutput")

    with tile.TileContext(nc) as tc:
        _emit(nc, tc, tile, mybir, io, debug)
    nc.compile()
    return nc


def _emit(nc, tc, tile, mybir, io, debug):
    f32 = mybir.dt.float32
    f32r = mybir.dt.float32r
    Alu = mybir.AluOpType
    Act = mybir.ActivationFunctionType
    AX = mybir.AxisListType
    RG = [list(range(NCORES))]

    def mm(outap, lhsT, rhs, start, stop, rep=True):
        if rep:
            lhsT = lhsT.bitcast(f32r)
            rhs = rhs.bitcast(f32r)
        nc.tensor.matmul(outap, lhsT, rhs, start=start, stop=stop,
                         skip_group_check=True)

    ctx = ExitStack()
    per = ctx.enter_context(tc.tile_pool(name="per", bufs=1))
    ps_c = ctx.enter_context(tc.tile_pool(name="ps_c", bufs=1, space="PSUM"))
    dram = ctx.enter_context(tc.tile_pool(name="dram", bufs=1, space="DRAM"))

    # ---------------- constants ----------------
    ident = per.tile([128, 128], f32, name="ident")
    nc.gpsimd.memset(ident[:], 1.0)
    nc.gpsimd.affine_select(ident[:], ident[:], pattern=[[1, 128]],
                            compare_op=Alu.is_equal, fill=0.0, base=0,
                            channel_multiplier=-1)
    ones1x128 = per.tile([1, 128], f32, name="ones1x128")
    nc.gpsimd.memset(ones1x128[:], 1.0)
    nc.scalar.activation(ones1x128.bitcast(f32r), ones1x128[:], Act.Identity)
    scr_warm = per.tile([1, 128], f32, name="scr_warm")
    nc.scalar.activation(scr_warm[:], ones1x128[:], Act.Relu)
    nc.scalar.activation(scr_warm[:], ones1x128[:], Act.Square)
    nc.scalar.activation(scr_warm[:], ones1x128[:], Act.Sqrt)
    ones128x1 = per.tile([128, 1], f32, name="ones128x1")
    nc.gpsimd.memset(ones128x1[:], 1.0)
    eps_col = per.tile([128, 1], f32, name="eps_col")
    nc.gpsimd.memset(eps_col[:], EPS)

    # ---------------- dummy collective (comm warm-up) ----------------
    warm_in = dram.tile([128, 1], f32, name="warm_in")
    warm_out = dram.tile([128, 1], f32, name="warm_out")
    nc.sync.dma_start(warm_in[:], ones128x1[:])
    nc.gpsimd.collective_compute("AllReduce", Alu.add, replica_groups=RG,
                                 ins=[warm_in.opt()], outs=[warm_out.opt()])

    # ---------------- load weights + vectors (host pre-laid-out) --------
    # x0 first on the SP queue: conv1 is the critical path
    conv_in_cm = tc.tile_pool(name="conv_in", bufs=1)
    conv_in = conv_in_cm.__enter__()
    x0 = conv_in.tile([3, NPOS], f32, name="x0")
    wraw_cm = tc.tile_pool(name="wraw", bufs=1)
    wraw = wraw_cm.__enter__()
    x0tmp_cm = tc.tile_pool(name="x0tmp", bufs=1)
    x0tmp = x0tmp_cm.__enter__()
    x0raw = x0tmp.tile([3, NPOS], f32, name="x0raw")
    nc.sync.dma_start(x0raw[:], io["x0h"].ap())
    nc.scalar.activation(x0.bitcast(f32r), x0raw[:], Act.Identity)

    vecs = per.tile([128, 32], f32, name="vecs")
    nc.sync.dma_start(vecs[:], io["vecs"].ap())
    b1_sb = vecs[:, 0:1]
    gamma1_sb = vecs[:, 1:2]
    beta1_sb = vecs[:, 2:3]
    b2_sb = vecs[:, 3:5]
    b3_sb = vecs[:, 5:9]
    gamma3_sb = vecs[:, 9:13]
    beta3_sb = vecs[:, 13:17]
    b4_sb = vecs[:, 17:20]
    bf_sb = vecs[:, 20:23]
    gammaf_sb = vecs[:, 23:26]
    betaf_sb = vecs[:, 26:29]
    betaaff_sb = vecs[:, 29:32]
    W1_sb = per.tile([128, 3], f32, name="W1_sb")
    nc.sync.dma_start(W1_sb[:], io["W1"].ap())
    W1T0 = per.tile([3, 128], f32, name="W1T0")
    nc.sync.dma_start(W1T0[:], io["W1Th"].ap())
    W1T = per.tile([3, 128], f32, name="W1T")
    nc.scalar.activation(W1T.bitcast(f32r), W1T0[:], Act.Identity)
    alpha_row = per.tile([1, C], f32, name="alpha_row")
    nc.sync.dma_start(alpha_row[:], io["alpha_row"].ap())

    # bulk weight loads on the scalar queue so they don't block the SP queue.
    # f32r matmult inputs must come from a rounding instruction, not a DMA:
    # round W2T/W3T/W4T/b4row through the Act engine right after the loads
    # (WfT is only used in plain-f32 matmuls, conv1 runs in plain f32).
    wst = wraw.tile([128, 18 * 128], f32, name="wst")
    W2T = per.tile([128, 2 * 128], f32, name="W2T")
    nc.scalar.dma_start(wst[:, 16 * 128:18 * 128], io["W2Th"].ap())
    nc.scalar.activation(W2T.bitcast(f32r), wst[:, 16 * 128:18 * 128],
                         Act.Identity)
    W3T = per.tile([128, 16 * 128], f32, name="W3T")
    nc.scalar.dma_start(wst[:, :16 * 128], io["W3Th"].ap())
    nc.scalar.activation(W3T.bitcast(f32r), wst[:, :16 * 128], Act.Identity)
    WfT = per.tile([128, 18 * 128], f32, name="WfT")
    nc.scalar.dma_start(WfT[:], io["WfTh"].ap())

    def wblk(wt, nr, kc, mo):
        return wt[:, (kc * nr + mo) * 128:(kc * nr + mo) * 128 + 128]

    def w4rhs(kc):  # pos-major rhs [128, 384] = blocks (kc, mo=0..2)
        return W4T[:, kc * 3 * 128:(kc * 3 + 3) * 128]

    def bn_scale_shift(var_ap, mu_ap, gam_ap, bet_ap, pref, n=1):
        std = per.tile([128, n], f32, name=pref + "_std")
        nc.scalar.activation(std[:], var_ap, Act.Sqrt, bias=eps_col[:])
        rstd = per.tile([128, n], f32, name=pref + "_rstd")
        nc.vector.reciprocal(rstd[:], std[:])
        sc = per.tile([128, n], f32, name=pref + "_sc")
        nc.vector.tensor_tensor(sc[:], rstd[:], gam_ap, op=Alu.mult)
        sh = per.tile([128, n], f32, name=pref + "_sh")
        nc.vector.tensor_tensor(sh[:], mu_ap, sc[:], op=Alu.mult)
        nc.vector.tensor_tensor(sh[:], bet_ap, sh[:], op=Alu.subtract)
        return sc, sh

    # ================ BN1 moments from global input ================
    bn1_cm = tc.tile_pool(name="bn1", bufs=1)
    bn1p = bn1_cm.__enter__()
    pgm = bn1p.tile([128, 768], f32, name="pgm")   # [128, (jj:8, i:32, c:3)]
    nc.gpsimd.dma_start(pgm[:], io["pg_full"].ap().rearrange(
        "(p jj) i c -> p (jj i c)", p=128).opt())
    mcols = bn1p.tile([128, 12], f32, name="mcols")
    pv = pgm.rearrange("p (j c) -> p j c", c=3)
    scr256 = bn1p.tile([128, 256], f32, name="scr256")
    for i in range(3):
        for j in range(3):
            nc.vector.scalar_tensor_tensor(
                scr256[:], pv[:, :, i], 1.0, pv[:, :, j],
                op0=Alu.mult, op1=Alu.mult,
                accum_out=mcols[:, 3 * i + j:3 * i + j + 1])
        nc.vector.tensor_reduce(mcols[:, 9 + i:10 + i], pv[:, :, i],
                                axis=AX.X, op=Alu.add)
    m12 = ps_c.tile([1, 12], f32, name="m12", tag="cps")
    mm(m12[:], ones128x1[:], mcols[:], True, True, rep=False)
    m12s = bn1p.tile([1, 12], f32, name="m12s")
    nc.scalar.activation(m12s[:], m12[:], Act.Identity, scale=1.0 / NTOT_POS)
    M2sb = bn1p.tile([3, 3], f32, name="M2sb")
    nc.gpsimd.dma_start(M2sb[:],
                        m12s[:1, :9].rearrange("1 (i j) -> 1 i j", i=3))
    mu3 = bn1p.tile([3, 1], f32, name="mu3")
    nc.gpsimd.dma_start(mu3[:], m12s[:1, 9:12])

    m1ps = ps_c.tile([128, 1], f32, name="m1ps", tag="cps")
    mm(m1ps[:], W1T[:], mu3[:], True, True, rep=False)   # W1 @ mu_p
    mvec = per.tile([128, 1], f32, name="mvec")
    nc.vector.tensor_copy(mvec[:], m1ps[:])
    wmps = ps_c.tile([128, 3], f32, name="wmps", tag="cps")
    mm(wmps[:], W1T[:], M2sb[:], True, True, rep=False)  # W1 @ M2
    # var of sc1*(W1 x + b1) is translation-invariant: var1 = E2raw - mvec^2
    # and the shift folds to sh1b = beta1 - sc1*mvec (b1 cancels)
    e2raw = per.tile([128, 1], f32, name="e2raw")
    scr3 = per.tile([128, 3], f32, name="scr3")
    nc.vector.scalar_tensor_tensor(scr3[:], wmps[:], 1.0, W1_sb[:],
                                   op0=Alu.mult, op1=Alu.mult,
                                   accum_out=e2raw[:])
    t_a = per.tile([128, 1], f32, name="t_a")
    nc.vector.tensor_tensor(t_a[:], mvec[:], mvec[:], op=Alu.mult)
    var1 = per.tile([128, 1], f32, name="var1")
    nc.vector.tensor_tensor(var1[:], e2raw[:], t_a[:], op=Alu.subtract)
    std1 = per.tile([128, 1], f32, name="std1")
    nc.scalar.activation(std1[:], var1[:], Act.Sqrt, bias=eps_col[:])
    rstd1 = per.tile([128, 1], f32, name="rstd1")
    nc.vector.reciprocal(rstd1[:], std1[:])
    sc1 = per.tile([128, 1], f32, name="sc1")
    nc.vector.tensor_tensor(sc1[:], rstd1[:], gamma1_sb[:], op=Alu.mult)
    sh1b = per.tile([128, 1], f32, name="sh1b")
    nc.vector.tensor_tensor(sh1b[:], mvec[:], sc1[:], op=Alu.mult)
    nc.vector.tensor_tensor(sh1b[:], beta1_sb[:], sh1b[:], op=Alu.subtract)
    bn1_cm.__exit__(None, None, None)
    x0tmp_cm.__exit__(None, None, None)

    # early DMAs for the selection phase (consumed later)
    selin_cm = tc.tile_pool(name="selin", bufs=1)
    selin = selin_cm.__enter__()
    pgA = selin.tile([128, 96], f32, name="pgA")
    nc.gpsimd.dma_start(pgA[:], io["pgAh"].ap())
    pgB = selin.tile([128, 3072], f32, name="pgB")
    for gp in range(4):
        nc.gpsimd.dma_start(
            pgB[32 * gp:32 * gp + 32],
            io["pgSh"].ap()[gp:gp + 1].broadcast_to([32, 3072]))


    # ---------------- data-independent H-phase constants ----------------
    # onesblk[32gp+n, gp'] = 1 iff gp' == gp; stationary for per-t U writes
    onesblk = per.tile([128, 4], f32, name="onesblk")
    nc.gpsimd.memset(onesblk[:], 1.0)
    nc.gpsimd.affine_select(onesblk[:], onesblk[:], pattern=[[-32, 4]],
                            compare_op=Alu.is_ge, fill=0.0, base=0,
                            channel_multiplier=1)
    nc.gpsimd.affine_select(onesblk[:], onesblk[:], pattern=[[32, 4]],
                            compare_op=Alu.is_ge, fill=0.0, base=31,
                            channel_multiplier=-1)
    alphar_ps = ps_c.tile([128, C], f32, name="alphar_ps", tag="cps")
    alpha_row = per.tile([1, C], f32, name="alpha_row")
    nc.sync.dma_start(alpha_row[:], io["alpha_row"].ap())

    # ================ conv1 / conv2 ================
    ps_mm_cm = tc.tile_pool(name="ps_mm", bufs=6, space="PSUM")
    ps_mm = ps_mm_cm.__enter__()
    act3_cm = tc.tile_pool(name="act3", bufs=1)
    act3 = act3_cm.__enter__()
    act1_cm = tc.tile_pool(name="act1", bufs=1)
    act1 = act1_cm.__enter__()

    f1h = act1.tile([128, NPOS], f32, name="f1h")
    for nt in range(8):
        ps = ps_mm.tile([128, 512], f32, name="mmps", tag="mmps")
        mm(ps[:], W1T[:], x0[:, nt * 512:(nt + 1) * 512], True, True)
        nc.scalar.activation(f1h[:, nt * 512:(nt + 1) * 512].bitcast(f32r),
                             ps[:], Act.Relu, bias=sh1b[:], scale=sc1[:])
    if debug:
        nc.sync.dma_start(io["dbg_f1h"].ap(), f1h[:])

    fg = per.tile([128, 256], f32, name="fg")  # [128, (mo:2, g:128)]
    f2 = [act3.tile([128, NPOS], f32, name=f"f2_{mo}") for mo in range(2)]
    for mo in range(2):
        for nt in range(8):
            ps = ps_mm.tile([128, 512], f32, name="mmps", tag="mmps")
            mm(ps[:], wblk(W2T, 2, 0, mo), f1h[:, nt * 512:(nt + 1) * 512],
               True, True)
            nc.scalar.activation(
                f2[mo][:, nt * 512:(nt + 1) * 512].bitcast(f32r), ps[:],
                Act.Identity, bias=b2_sb[:, mo:mo + 1])
            # per-group max per chunk (16 groups) so fg is ready for conv3
            nc.vector.tensor_reduce(
                fg[:, mo * 128 + nt * 16:mo * 128 + (nt + 1) * 16]
                .bitcast(f32r),
                f2[mo][:, nt * 512:(nt + 1) * 512]
                .rearrange("p (g i) -> p g i", i=32),
                axis=AX.X, op=Alu.max)
    act1_cm.__exit__(None, None, None)
    if debug:
        nc.sync.dma_start(io["dbg_fg"].ap(), fg[:])

    # ================ conv3 (stats in, bias copy out) ================
    f3 = [per.tile([128, NPOS], f32, name=f"f3_{mo}") for mo in range(4)]
    stats3 = per.tile([128, 4 * 8 * 6], f32, name="stats3")
    mv3 = per.tile([128, 8], f32, name="mv3")
    for mo in range(4):
        for ntc in range(4):
            pss = [ps_mm.tile([128, 512], f32, name="mmps", tag="mmps")
                   for _ in range(2)]
            # f2 blocks first so the fg reduce is off the critical path
            for kc in (2, 3, 0, 1):
                for j, nt in enumerate((2 * ntc, 2 * ntc + 1)):
                    if kc < 2:
                        rhs = fg[:, kc * 128 + nt * 16:
                                 kc * 128 + (nt + 1) * 16] \
                            .unsqueeze(2).broadcast_to([128, 16, 32])
                    else:
                        rhs = f2[kc - 2][:, nt * 512:(nt + 1) * 512]
                    mm(pss[j][:], wblk(W3T, 4, kc, mo), rhs, kc == 2,
                       kc == 1)
            for j, nt in enumerate((2 * ntc, 2 * ntc + 1)):
                dst = f3[mo][:, nt * 512:(nt + 1) * 512].bitcast(f32r)
                nc.scalar.activation(dst, pss[j][:], Act.Identity,
                                     bias=b3_sb[:, mo:mo + 1])
                nc.vector.bn_stats(
                    stats3[:, (mo * 8 + nt) * 6:(mo * 8 + nt) * 6 + 6],
                    dst)
        nc.vector.bn_aggr(mv3[:, mo * 2:mo * 2 + 2],
                          stats3[:, mo * 48:(mo + 1) * 48])
    act3_cm.__exit__(None, None, None)
    ps_mm_cm.__exit__(None, None, None)

    # local (sum, sumsq) per channel -> AllReduce (launch ASAP)
    # psum stats lack +b3, but b3 cancels in the variance; ship raw
    # sums/sumsq and add b3 to the global mean after the AllReduce
    bnloc = per.tile([128, 8], f32, name="bnloc")
    mv3v = mv3.rearrange("p (m two) -> p two m", two=2)
    bnlv = bnloc.rearrange("p (m two) -> p two m", two=2)
    nc.vector.tensor_scalar(bnlv[:, 0, :], mv3v[:, 0, :], float(NPOS), None,
                            op0=Alu.mult)
    scrb3 = per.tile([128, 4], f32, name="scrb3")
    nc.vector.scalar_tensor_tensor(scrb3[:], mv3v[:, 0, :], 1.0,
                                   mv3v[:, 0, :], op0=Alu.mult, op1=Alu.mult)
    nc.vector.tensor_tensor(scrb3[:], scrb3[:], mv3v[:, 1, :], op=Alu.add)
    nc.vector.tensor_scalar(bnlv[:, 1, :], scrb3[:], float(NPOS), None,
                            op0=Alu.mult)
    cc3_in = dram.tile([128, 8], f32, name="cc3_in")
    cc3_out = dram.tile([128, 8], f32, name="cc3_out")
    nc.sync.dma_start(cc3_in[:], bnloc[:])
    nc.gpsimd.collective_compute("AllReduce", Alu.add, replica_groups=RG,
                                 ins=[cc3_in.opt()], outs=[cc3_out.opt()])

    W4T = per.tile([128, 12 * 128], f32, name="W4T")
    nc.scalar.dma_start(wst[:, :12 * 128], io["W4Th"].ap())
    nc.scalar.activation(W4T.bitcast(f32r), wst[:, :12 * 128], Act.Identity)
    b4row0 = per.tile([1, C], f32, name="b4row0")
    nc.sync.dma_start(b4row0[:], io["b4row"].ap())
    b4row = per.tile([1, C], f32, name="b4row")
    nc.scalar.activation(b4row.bitcast(f32r), b4row0[:], Act.Identity)
    # wbias = Wf[:, C:] @ beta_aff + bf   (channel-major [128, 3])
    wbias_ps = ps_c.tile([128, 3], f32, name="wbias_ps", tag="cps")
    for mo in range(3):
        for kc in range(3):
            mm(wbias_ps[:, mo:mo + 1], wblk(WfT, 3, 3 + kc, mo),
               betaaff_sb[:, kc:kc + 1], kc == 0, kc == 2, rep=False)
    wbias = per.tile([128, 3], f32, name="wbias")
    nc.vector.tensor_tensor(wbias[:], wbias_ps[:], bf_sb[:], op=Alu.add)

    mm(alphar_ps[:], ones1x128[:], alpha_row[:], True, True, rep=False)
    alphar = per.tile([128, C], f32, name="alphar")
    nc.scalar.activation(alphar[:], alphar_ps[:], Act.Identity)


    # ====== selection, declared here so it runs inside the BN3 window ======
    hconst_cm = tc.tile_pool(name="hconst", bufs=1)
    hc = hconst_cm.__enter__()
    # W_B zero background (block-diag A2T copied in below)
    W_B = hc.tile([128, NPOS], f32, name="W_B")
    nc.gpsimd.memset(W_B[:], 0.0)
    nc.gpsimd.tensor_copy(W_B.bitcast(f32r), W_B[:])
    # onesU[32*gp+n, t*128 + m] = 1 iff m == 4t+gp
    onesU = hc.tile([128, 32 * 128], f32, name="onesU")
    nc.gpsimd.memset(onesU[:], 0.0)
    nc.gpsimd.tensor_copy(onesU.bitcast(f32r), onesU[:])
    for t in range(32):
        nc.gpsimd.tensor_copy(
            onesU[:, t * 128 + 4 * t:t * 128 + 4 * t + 4].bitcast(f32r),
            onesblk[:])
    sel_b = tc.tile_pool(name="sel_b", bufs=1)
    sb = sel_b.__enter__()
    # negkey[32gp+n, t*32+m] = sum_c (pgA[.,t,c] - 0.5*pgB_c)*pgB_c
    scr1 = sb.tile([128, 1024], f32, name="scr1")
    negkey = sb.tile([128, 1024], f32, name="negkey")
    for cdim in range(3):
        pgB_c = pgB.rearrange("p (t m c) -> p t m c", t=32, m=32)[:, :, :, cdim]
        pgA_c = pgA.rearrange("p (t c) -> p t c", c=3)[:, :, cdim] \
            .unsqueeze(2).broadcast_to([128, 32, 32])
        dst = scr1[:] if cdim > 0 else negkey[:]
        dstv = dst.rearrange("p (t m) -> p t m", t=32)
        nc.vector.scalar_tensor_tensor(dstv, pgB_c, -0.5, pgA_c,
                                       op0=Alu.mult, op1=Alu.add)
        nc.vector.tensor_tensor(dstv, dstv, pgB_c, op=Alu.mult)
        if cdim > 0:
            nc.vector.tensor_tensor(negkey[:], negkey[:], scr1[:],
                                    op=Alu.add)

    top8 = sb.tile([128, 8], f32, name="top8")
    repl = sb.tile([128, 1024], f32, name="repl", tag="repl")
    for t in range(32):
        nc.vector.max(top8[:], negkey[:, t * 32:(t + 1) * 32])
        nc.vector.match_replace(repl[:, t * 32:(t + 1) * 32], top8[:],
                                negkey[:, t * 32:(t + 1) * 32], BIG_NEG)
    A2 = sb.tile([128, 1024], f32, name="A2")
    nc.vector.tensor_scalar(A2[:], repl[:], BIG_NEG, None, op0=Alu.is_equal)
    if debug:
        nc.sync.dma_start(io["dbg_negkey"].ap(), negkey[:])
        nc.sync.dma_start(io["dbg_A2"].ap(), A2[:])

    A2T = sb.tile([128, 1024], f32, name="A2T", tag="repl")
    nc.vector.transpose(A2T[:], A2[:])
    # Kc[32gp+m, t] = K + sum_n A[n, m];  Kw = Kc - 2K (U weights)
    Kc = per.tile([128, 32], f32, name="Kc")
    nc.vector.tensor_reduce(Kc[:],
                            A2T.rearrange("p (t n) -> p t n", t=32),
                            axis=AX.X, op=Alu.add)
    nc.vector.tensor_scalar(Kc[:], Kc[:], float(K), None, op0=Alu.add)
    Kw = per.tile([128, 32], f32, name="Kw")
    nc.vector.tensor_scalar(Kw[:], Kc[:], -2.0 * K, None, op0=Alu.add)
    if debug:
        nc.sync.dma_start(io["dbg_Kc"].ap(), Kc[:])

    # W_B[32gp+m, t*128+32gp+n] = A2T[32gp+m, t*32+n]  (block-diag lhsT)
    for gp in range(4):
        nc.vector.tensor_copy(
            W_B[32 * gp:32 * gp + 32].rearrange(
                "p (t q) -> p t q", t=32)[:, :, 32 * gp:32 * gp + 32]
            .bitcast(f32r),
            A2T[32 * gp:32 * gp + 32].rearrange("p (t n) -> p t n", t=32))
    sel_b.__exit__(None, None, None)

    # ====== BN3 post-collective scale/shift + chunked ReLU3 ======
    g3 = per.tile([128, 8], f32, name="g3")
    nc.sync.dma_start(g3[:], cc3_out[:])
    gmu3 = per.tile([128, 4], f32, name="gmu3")
    nc.scalar.activation(gmu3[:], g3.rearrange("p (m two) -> p two m",
                                               two=2)[:, 0, :],
                         Act.Identity, scale=1.0 / NTOT_POS)
    ge23 = per.tile([128, 4], f32, name="ge23")
    nc.scalar.activation(ge23[:], g3.rearrange("p (m two) -> p two m",
                                               two=2)[:, 1, :],
                         Act.Identity, scale=1.0 / NTOT_POS)
    gvar3 = per.tile([128, 4], f32, name="gvar3")
    nc.vector.tensor_tensor(gvar3[:], gmu3[:], gmu3[:], op=Alu.mult)
    nc.vector.tensor_tensor(gvar3[:], ge23[:], gvar3[:], op=Alu.subtract)
    sc3, sh3 = bn_scale_shift(gvar3[:], gmu3[:], gamma3_sb[:], beta3_sb[:],
                              "bn3", n=4)

    # ReLU3 chunk nt covers H iterations t in [4nt, 4nt+4); interleave the
    # chunks into the H loop so the in-order Act queue doesn't drain all of
    # ReLU3 before feat t=0
    def relu3_chunk(nt, eng="pool"):
        for mo in range(4):
            sl = f3[mo][:, nt * 512:(nt + 1) * 512]
            if eng == "act":
                nc.scalar.activation(sl.bitcast(f32r), sl, Act.Relu,
                                     bias=sh3[:, mo:mo + 1],
                                     scale=sc3[:, mo:mo + 1])
            else:
                nc.gpsimd.tensor_scalar(sl.bitcast(f32r), sl,
                                        sc3[:, mo:mo + 1], sh3[:, mo:mo + 1],
                                        op0=Alu.mult, op1=Alu.add)
                nc.gpsimd.tensor_scalar(sl.bitcast(f32r), sl.bitcast(f32r),
                                        0.0, None, op0=Alu.max)

    # ================ H phase: conv4 pos-major, U, moments, fg2 =========
    fg2 = per.tile([128, C], f32, name="fg2")     # [128ch, (mo:3, g:128)]
    fg2v = fg2.rearrange("p (mo g) -> p mo g", mo=3)
    sqcol = per.tile([128, 32], f32, name="sqcol")  # ||feat_pos||^2 per t
    acc2 = per.tile([128, 32], f32, name="acc2")    # feat . h per t
    scrSq = per.tile([128, C], f32, name="scrSq")
    scrH = per.tile([128, C], f32, name="scrH")
    U_sb = per.tile([128, C], f32, name="U_sb")
    t1col = per.tile([128, 1], f32, name="t1col")

    with tc.tile_pool(name="psU", bufs=1, space="PSUM") as psU:
        Ups = psU.tile([128, C], f32, name="Ups", tag="hold")
        with tc.tile_pool(name="featpool", bufs=4) as featpool, \
             tc.tile_pool(name="psF", bufs=3, space="PSUM") as psF, \
             tc.tile_pool(name="psT", bufs=2, space="PSUM") as psT:
            for mo in range(2):
                sl = f3[mo][:, 0:512]
                nc.scalar.activation(sl.bitcast(f32r), sl, Act.Relu,
                                     bias=sh3[:, mo:mo + 1],
                                     scale=sc3[:, mo:mo + 1])
            for mo in range(2, 4):
                sl = f3[mo][:, 0:512]
                nc.vector.tensor_scalar(sl.bitcast(f32r), sl,
                                        sc3[:, mo:mo + 1], sh3[:, mo:mo + 1],
                                        op0=Alu.mult, op1=Alu.add)
                nc.vector.tensor_scalar(sl.bitcast(f32r), sl.bitcast(f32r),
                                        0.0, None, op0=Alu.max)
            relu3_chunk(1)
            for t in range(32):
                fps = psF.tile([128, C], f32, name="fps", tag="fps")
                for kc in range(4):
                    mm(fps[:], f3[kc][:, t * 128:(t + 1) * 128], w4rhs(kc),
                       kc == 0, False)
                mm(fps[:], ones1x128[:], b4row[:], False, True)  # + b4
                feat = featpool.tile([128, C], f32, name="feat", tag="feat")
                nc.scalar.activation(feat.bitcast(f32r), fps[:], Act.Identity)
                if debug and t == 0:
                    nc.sync.dma_start(io["dbg_feat0"].ap(), feat[:])
                # ||feat||^2 per position (from SBUF so fps frees earlier)
                nc.scalar.activation(scrSq[:], feat[:], Act.Square,
                                     accum_out=sqcol[:, t:t + 1])
                # fg2 via PE transpose (f32r) + combined per-group max
                pst = psT.tile([128, C], f32, name="tps", tag="tps")
                for mo in range(3):
                    nc.tensor.matmul(
                        pst[:, mo * 128:(mo + 1) * 128],
                        feat[:, mo * 128:(mo + 1) * 128],
                        ident[:], is_transpose=True,
                        skip_group_check=True)
                nc.vector.tensor_reduce(
                    fg2v[:, :, 4 * t:4 * t + 4],
                    pst.rearrange("p (mo gp i) -> p mo gp i", mo=3, i=32),
                    axis=AX.X, op=Alu.max)
                # neighbor sum (for the T2 moment only)
                hps = psF.tile([128, C], f32, name="hps", tag="hps",
                               bufs=1)
                mm(hps[:], W_B[:, t * 128:(t + 1) * 128], feat[:],
                   True, True)
                nc.vector.scalar_tensor_tensor(
                    scrH[:], feat[:], 1.0, hps[:],
                    op0=Alu.mult, op1=Alu.mult, accum_out=acc2[:, t:t + 1])
                # U path: wfeat = (Kc - 2K) * feat
                wfeat = featpool.tile([128, C], f32, name="wfeat", tag="wf")
                nc.vector.tensor_scalar(wfeat.bitcast(f32r), feat[:],
                                        Kw[:, t:t + 1], None, op0=Alu.mult)
                mm(Ups[:], onesU[:, t * 128:(t + 1) * 128], wfeat[:],
                   t == 0, t == 31)
                if t % 4 == 0 and t // 4 + 2 < 8:
                    nt = t // 4 + 2
                    relu3_chunk(nt, eng="act" if nt % 2 == 1 else "pool")
        nc.scalar.activation(U_sb[:], Ups[:], Act.Identity,
                             accum_out=t1col[:])
    hconst_cm.__exit__(None, None, None)
    selin_cm.__exit__(None, None, None)
    wraw_cm.__exit__(None, None, None)
    conv_in_cm.__exit__(None, None, None)
    if debug:
        nc.sync.dma_start(io["dbg_fg2"].ap(), fg2[:])

    # t2col = sum_t (Kc*sq) - 2*sum_t acc2
    a1r = per.tile([128, 1], f32, name="a1r")
    scr32 = per.tile([128, 32], f32, name="scr32")
    nc.vector.scalar_tensor_tensor(scr32[:], sqcol[:], 1.0, Kc[:],
                                   op0=Alu.mult, op1=Alu.mult,
                                   accum_out=a1r[:])
    a2r = per.tile([128, 1], f32, name="a2r")
    nc.vector.tensor_reduce(a2r[:], acc2[:], axis=AX.X, op=Alu.add)
    t2col = per.tile([128, 1], f32, name="t2col")
    nc.vector.scalar_tensor_tensor(t2col[:], a2r[:], -2.0, a1r[:],
                                   op0=Alu.mult, op1=Alu.add)

    # V = alpha * U / (n*K)  (group-major), then transpose to channel-major
    V_sb = per.tile([128, C], f32, name="V_sb")
    nc.vector.scalar_tensor_tensor(V_sb[:], U_sb[:], 1.0 / (NPTS * K),
                                   alphar[:], op0=Alu.mult, op1=Alu.mult)
    Vc = per.tile([128, C], f32, name="Vc")
    for mo in range(3):
        pstv = ps_c.tile([128, 128], f32, name="wtps", tag="cps")
        nc.tensor.transpose(pstv[:], V_sb[:, mo * 128:(mo + 1) * 128],
                            ident[:])
        nc.vector.tensor_copy(Vc[:, mo * 128:(mo + 1) * 128], pstv[:])
    if debug:
        nc.sync.dma_start(io["dbg_U"].ap(), U_sb[:])

    # ================ P/Q matmuls + moments ================
    P_sb = per.tile([128, C], f32, name="P_sb")
    Q_sb = per.tile([128, C], f32, name="Q_sb")
    mom = per.tile([128, 17], f32, name="mom")
    scrP = per.tile([128, 128], f32, name="scrP")
    with tc.tile_pool(name="psQ", bufs=1, space="PSUM") as psQ:
        Pps = psQ.tile([128, C], f32, name="Pps", tag="holdP")
        Qps = psQ.tile([128, C], f32, name="Qps", tag="holdQ")
        for mo in range(3):
            for kc in range(3):
                mm(Pps[:, mo * 128:(mo + 1) * 128], wblk(WfT, 3, kc, mo),
                   fg2[:, kc * 128:(kc + 1) * 128], kc == 0, kc == 2,
                   rep=False)
        for mo in range(3):
            for kc in range(3):
                mm(Qps[:, mo * 128:(mo + 1) * 128], wblk(WfT, 3, 3 + kc, mo),
                   Vc[:, kc * 128:(kc + 1) * 128], kc == 0, kc == 2,
                   rep=False)
        scrQ = per.tile([128, 128], f32, name="scrQ")
        scrPQ = per.tile([128, 128], f32, name="scrPQ")
        for mo in range(3):
            nc.scalar.activation(P_sb[:, mo * 128:(mo + 1) * 128],
                                 Pps[:, mo * 128:(mo + 1) * 128],
                                 Act.Identity, bias=wbias[:, mo:mo + 1],
                                 accum_out=mom[:, mo:mo + 1])
            nc.scalar.activation(Q_sb[:, mo * 128:(mo + 1) * 128],
                                 Qps[:, mo * 128:(mo + 1) * 128], Act.Identity,
                                 accum_out=mom[:, 3 + mo:4 + mo])
            nc.scalar.activation(scrP[:], P_sb[:, mo * 128:(mo + 1) * 128],
                                 Act.Square, accum_out=mom[:, 6 + mo:7 + mo])
            nc.vector.scalar_tensor_tensor(
                scrQ[:], Q_sb[:, mo * 128:(mo + 1) * 128], 1.0,
                Q_sb[:, mo * 128:(mo + 1) * 128], op0=Alu.mult, op1=Alu.mult,
                accum_out=mom[:, 9 + mo:10 + mo])
            nc.vector.scalar_tensor_tensor(
                scrPQ[:], P_sb[:, mo * 128:(mo + 1) * 128], 1.0,
                Q_sb[:, mo * 128:(mo + 1) * 128], op0=Alu.mult, op1=Alu.mult,
                accum_out=mom[:, 12 + mo:13 + mo])
    # partition-sum t1/t2 and broadcast before the AllReduce so gmom holds
    # the global scalars directly on every partition
    t12l = per.tile([128, 2], f32, name="t12l")
    nc.vector.tensor_copy(t12l[:, 0:1], t1col[:])
    nc.vector.tensor_copy(t12l[:, 1:2], t2col[:])
    t12_ps = ps_c.tile([1, 2], f32, name="t12_ps", tag="cps")
    mm(t12_ps[:], ones128x1[:], t12l[:], True, True, rep=False)
    t12 = per.tile([1, 2], f32, name="t12")
    nc.vector.tensor_copy(t12[:], t12_ps[:])
    t12b_ps = ps_c.tile([128, 2], f32, name="t12b_ps", tag="cps")
    mm(t12b_ps[:], ones1x128[:], t12[:], True, True, rep=False)
    nc.vector.tensor_copy(mom[:, 15:17], t12b_ps[:])
    if debug:
        nc.sync.dma_start(io["dbg_P"].ap(), P_sb[:])
        nc.sync.dma_start(io["dbg_Q"].ap(), Q_sb[:])
        nc.sync.dma_start(io["dbg_mom"].ap(), mom[:])

    ccf_in = dram.tile([128, 17], f32, name="ccf_in")
    ccf_out = dram.tile([128, 17], f32, name="ccf_out")
    nc.sync.dma_start(ccf_in[:], mom[:])
    nc.gpsimd.collective_compute("AllReduce", Alu.add, replica_groups=RG,
                                 ins=[ccf_in.opt()], outs=[ccf_out.opt()])
    gmom = per.tile([128, 17], f32, name="gmom")
    nc.sync.dma_start(gmom[:], ccf_out[:])

    T1 = gmom[:, 15:16]
    T2 = gmom[:, 16:17]

    # s = 1 / (std + EPS); var = (T2 - T1^2/N) / (N-1)
    tA = per.tile([128, 1], f32, name="tA")
    nc.vector.tensor_tensor(tA[:], T1, T1, op=Alu.mult)
    tB = per.tile([128, 1], f32, name="tB")
    nc.vector.scalar_tensor_tensor(tB[:], tA[:], -1.0 / NTOT_DX, T2,
                                   op0=Alu.mult, op1=Alu.add)
    stdx = per.tile([128, 1], f32, name="stdx")
    nc.scalar.activation(stdx[:], tB[:], Act.Sqrt,
                         scale=1.0 / (NTOT_DX - 1))
    nc.vector.tensor_scalar(stdx[:], stdx[:], EPS, None, op0=Alu.add)
    s_col = per.tile([128, 1], f32, name="s_col")
    nc.vector.reciprocal(s_col[:], stdx[:])
    s2_col = per.tile([128, 1], f32, name="s2_col")
    nc.vector.tensor_tensor(s2_col[:], s_col[:], s_col[:], op=Alu.mult)
    ts2 = per.tile([128, 1], f32, name="ts2")
    nc.vector.tensor_scalar(ts2[:], s_col[:], 2.0, None, op0=Alu.mult)

    # ================ BNf + output (vectorized over mo) ================
    # muf = (sumP + s*sumQ) / NB
    muf = per.tile([128, 3], f32, name="muf")
    nc.vector.scalar_tensor_tensor(muf[:], gmom[:, 3:6], s_col[:],
                                   gmom[:, 0:3], op0=Alu.mult, op1=Alu.add)
    nc.scalar.activation(muf[:], muf[:], Act.Identity, scale=1.0 / NB)
    # e2f = (sumP2 + 2s*sumPQ + s^2*sumQ2) / NB
    e2f = per.tile([128, 3], f32, name="e2f")
    nc.vector.scalar_tensor_tensor(e2f[:], gmom[:, 12:15], ts2[:],
                                   gmom[:, 6:9], op0=Alu.mult, op1=Alu.add)
    nc.vector.scalar_tensor_tensor(e2f[:], gmom[:, 9:12], s2_col[:],
                                   e2f[:], op0=Alu.mult, op1=Alu.add)
    nc.scalar.activation(e2f[:], e2f[:], Act.Identity, scale=1.0 / NB)
    varf = per.tile([128, 3], f32, name="varf")
    nc.vector.tensor_tensor(varf[:], muf[:], muf[:], op=Alu.mult)
    nc.vector.tensor_tensor(varf[:], e2f[:], varf[:], op=Alu.subtract)
    scf, shf = bn_scale_shift(varf[:], muf[:], gammaf_sb[:], betaf_sb[:],
                              "bnf", n=3)
    outsb = per.tile([128, C], f32, name="outsb")
    zf = per.tile([128, C], f32, name="zf")
    nc.vector.scalar_tensor_tensor(zf[:], Q_sb[:], s_col[:], P_sb[:],
                                   op0=Alu.mult, op1=Alu.add)
    for mo in range(3):
        fused = per.tile([128, 128], f32, name=f"fused_{mo}")
        nc.scalar.activation(fused[:], zf[:, mo * 128:(mo + 1) * 128],
                             Act.Relu, bias=shf[:, mo:mo + 1],
                             scale=scf[:, mo:mo + 1])
        pst = ps_c.tile([128, 128], f32, name="wtps", tag="cps")
        nc.tensor.transpose(pst[:], fused[:], ident[:])
        nc.vector.tensor_copy(outsb[:, mo * 128:(mo + 1) * 128], pst[:])

    nc.sync.dma_start(io["out"].ap(), outsb[:])
    ctx.close()


def _get_built():
    global _BUILT
    if _BUILT is None:
        _BUILT = _build(DEBUG)
    return _BUILT


def _wt_host(W, rows, cols):
    """Host layout matching wblk(): out[q, (kc*nr+mo)*128+p] =
    W[mo*128+p, kc*128+q]."""
    nr, kg = rows // 128, cols // 128
    blk = W.reshape(nr, 128, kg, 128)          # [mo, p, kc, q]
    return np.ascontiguousarray(
        blk.transpose(3, 2, 0, 1).reshape(128, kg * nr * 128))


def _vec_host(v):
    n = v.shape[0]
    nt = n // 128
    return np.ascontiguousarray(v.reshape(nt, 128).T)


def make_in_maps(inputs):
    pgf = np.ascontiguousarray(
        np.asarray(inputs["point_groups"], dtype=np.float32).reshape(
            1024, NPTS, 3))
    f = lambda n: np.asarray(inputs[n], dtype=np.float32)
    base = {
        "pg_full": pgf,
        "W1": np.ascontiguousarray(f("W1")),
        "W1Th": np.ascontiguousarray(f("W1").T),
        "W2Th": _wt_host(f("W2"), 256, 128),
        "W3Th": _wt_host(f("W3"), 512, 512),
        "W4Th": _wt_host(f("W4"), C, 512),
        "WfTh": _wt_host(f("Wf"), C, 2 * C),
        "vecs": np.ascontiguousarray(np.concatenate(
            [_vec_host(f("b1")), _vec_host(f("gamma1")),
             _vec_host(f("beta1")), _vec_host(f("b2")),
             _vec_host(f("b3")), _vec_host(f("gamma3")),
             _vec_host(f("beta3")), _vec_host(f("b4")),
             _vec_host(f("bf")), _vec_host(f("gammaf")),
             _vec_host(f("betaf")),
             _vec_host(f("beta_aff").reshape(C))], axis=1)),
        "b4row": np.ascontiguousarray(f("b4").reshape(1, C)),
        "alpha_row": np.ascontiguousarray(f("alpha").reshape(1, C)),
    }
    in_maps = []
    for c in range(NCORES):
        m = dict(base)
        pg = pgf[c * G:(c + 1) * G]            # [(t gp), i, c]
        pgr = pg.reshape(32, 4, NPTS, 3)       # [t, gp, i, c]
        m["x0h"] = np.ascontiguousarray(
            pgr.transpose(3, 0, 1, 2).reshape(3, NPOS))
        m["pgAh"] = np.ascontiguousarray(
            pgr.transpose(1, 2, 0, 3).reshape(128, 96))
        m["pgSh"] = np.ascontiguousarray(
            pgr.transpose(1, 0, 2, 3).reshape(4, NPTS * 32 * 3))
        in_maps.append(m)
    return in_maps


def kernel(**inputs):
    from concourse.bass_utils import run_bass_kernel_spmd

    nc = _get_built()
    in_maps = make_in_maps(inputs)
    res = run_bass_kernel_spmd(nc, in_maps, list(range(NCORES)))
    full = np.concatenate([res.results[c]["out"] for c in range(NCORES)],
                          axis=0)
    return full.reshape(4, 256, C)
